# revision 1
# baseline (speedup 1.0000x reference)
"""Trainium2 Bass kernel for nn_Block_4526895530469 (Mamba block + MLP residual).

Sharding over 8 NeuronCores: core c -> batch b=c//4, channel shard r=c%4
(512 of the 2048 d_inner channels), full T=2048 sequence per core. The
selective scan runs full-T per channel on the Vector engine
(tensor_tensor_scan), so there is no cross-core state chain. Layout for the
scan is (s,e)-pairs on partitions (row p = 8*s + e_local, 16 states x 8
channels per 128-row tile) x time on the free dimension.

Collectives: one zero-padded global AllReduce for the (96, T) x_dbl partial
sums (contraction over the sharded d_inner), and one 4-group ReduceScatter
for the out_proj partials which simultaneously scatters tokens for the
token-parallel MLP tail.
"""
import sys
sys.path.insert(0, '/opt/trn_rl_repo')

import numpy as np
from contextlib import ExitStack

import concourse.bass as bass
from concourse import bacc
import concourse.tile as tile
from concourse import mybir
from concourse.bass_utils import run_bass_kernel_spmd

# The interp (used by Tile's scheduling pass and by test simulation) lacks
# Silu; emulate it: run the existing Sigmoid path, then multiply by the
# scaled/biased input.
from concourse import bass_interp as _bi
from concourse import mybir as _mb

_orig_visit_act = _bi.InstructionExecutor.visit_InstActivation


def _visit_act_with_silu(self, instruction, *a, **kw):
    if instruction.func != _mb.ActivationFunctionType.Silu:
        return _orig_visit_act(self, instruction, *a, **kw)
    import numpy as _np
    assert len(instruction.outs) == 1, "Silu shim: no accum_out support"
    func0 = instruction.func
    try:
        instruction.func = _mb.ActivationFunctionType.Sigmoid
        res = _orig_visit_act(self, instruction, *a, **kw)
    finally:
        instruction.func = func0
    reg_snapshot = kw.get("reg_snapshot")
    inp = self.view_ap(instruction.ins[0], _bi.Direction.READ, instruction,
                       reg_snapshot=reg_snapshot).astype(_np.float32)
    inp = inp.reshape(inp.shape[0], -1)

    def _val(arg):
        if isinstance(arg, _mb.ImmediateValue):
            return arg.value
        v = self.view_ap(arg, _bi.Direction.READ, instruction,
                         reg_snapshot=reg_snapshot).astype(_np.float32)
        return v.reshape(v.shape[0], -1)

    bias = _val(instruction.ins[1])
    scale = _val(instruction.ins[2])
    sx = inp * scale + bias
    out_view = self.view_ap(instruction.outs[0], _bi.Direction.WRITE, instruction,
                            reg_snapshot=reg_snapshot)
    sig = _np.asarray(out_view, dtype=_np.float32).reshape(sx.shape)
    out_view[:] = (sig * sx).reshape(out_view.shape).astype(out_view.dtype)
    return res


_bi.InstructionExecutor.visit_InstActivation = _visit_act_with_silu

F32 = mybir.dt.float32
BF16 = mybir.dt.bfloat16
AF = mybir.ActivationFunctionType
ALU = mybir.AluOpType

D_MODEL, D_INNER, D_STATE, D_CONV, DT_RANK = 1024, 2048, 16, 4, 64
B, T = 2, 2048
EL = D_INNER // 4          # 512 channels per core
NET = EL // 128            # 4 e-tiles
NJ = EL // 8               # 64 scan tiles
NCH = T // 512             # 4 t-chunks
TQ = T // 4                # 512 tokens for the MLP tail
XD = DT_RANK + 2 * D_STATE  # 96
EPS = float(np.finfo(np.float32).eps)

_CACHE = {}


def _build(nocc=False, gps_mod=2, ar_bf16=True, nocc_ar=False, nocc_rs=False, rs_split=True):
    nc = bacc.Bacc("TRN2", target_bir_lowering=False, debug=False, num_devices=8)

    def din(name, shape, dt=BF16):
        return nc.dram_tensor(name, list(shape), dt, kind="ExternalInput").ap()

    xb = din("xb", (T, D_MODEL), F32)
    xq = din("xq", (TQ, D_MODEL), F32)
    w_in_T = din("w_in_T", (D_MODEL, 2 * EL))
    conv_wc = din("conv_wc", (128, NET * D_CONV), F32)   # cols [4k:4k+4] = e-tile k
    conv_bc = din("conv_bc", (128, NET), F32)
    w_xp_T = din("w_xp_T", (EL, XD))
    w_dt_T = din("w_dt_T", (DT_RANK, EL))
    dt_bnc = din("dt_bnc", (128, NET), F32)              # -dt_proj_b
    negA = din("negA", (128, NJ), F32)                   # exp(A_log) per scan tile col
    d_c = din("d_c", (128, NET), F32)
    w_out_T = din("w_out_T", (EL, D_MODEL))
    w_fc_T = din("w_fc_T", (D_MODEL, 2 * D_MODEL))
    w_pr_T = din("w_pr_T", (2 * D_MODEL, D_MODEL))
    r01 = din("r01", (16 * 128, 128))
    g01 = din("g01", (16 * 128, 128))
    s01n = din("s01n", (D_STATE, 128))
    s01p = din("s01p", (D_STATE, 128))
    ident_bf = din("ident_bf", (128, 128))
    ident_f32 = din("ident_f32", (128, 128), F32)
    ones_bf = din("ones_bf", (128, 1))
    ones_row_bf = din("ones_row_bf", (1, 128))
    msk0 = din("msk0", (XD, 1), F32)
    msk1 = din("msk1", (XD, 1), F32)

    out = nc.dram_tensor("out", [TQ, D_MODEL], F32, kind="ExternalOutput").ap()

    xdbl_in = nc.dram_tensor("xdbl_in", [2 * XD, T], BF16).ap()
    xdbl_out = nc.dram_tensor("xdbl_out", [2 * XD, T], BF16).ap()
    rs_in_h = [nc.dram_tensor(f"rs_in{h}", [4 * D_MODEL, TQ // 2], BF16).ap() for h in range(2)]
    rs_out_h = [nc.dram_tensor(f"rs_out{h}", [D_MODEL, TQ // 2], BF16).ap() for h in range(2)]
    dtu_dram = nc.dram_tensor("dtu_dram", [EL, T], BF16).ap()
    xqT_dram = nc.dram_tensor("xqT_dram", [D_MODEL, TQ], F32).ap()

    g8 = [[0, 1, 2, 3, 4, 5, 6, 7]]
    g4 = [[0, 1, 2, 3], [4, 5, 6, 7]]

    with tile.TileContext(nc) as tc, ExitStack() as top:
        cpool = top.enter_context(tc.tile_pool(name="consts", bufs=1))

        def cload(nm, name_ap, shape, dt=BF16):
            t = cpool.tile(list(shape), dt, tag=nm, name=nm)
            nc.sync.dma_start(t[:], name_ap)
            return t

        negA_t = cload("negA_t", negA, (128, NJ), F32)
        convw_t = cload("convw_t", conv_wc, (128, NET * D_CONV), F32)
        convb_t = cload("convb_t", conv_bc, (128, NET), F32)
        dtbn_t = cload("dtbn_t", dt_bnc, (128, NET), F32)
        dcol_t = cload("dcol_t", d_c, (128, NET), F32)
        s01n_t = cload("s01n_t", s01n, (D_STATE, 128))
        s01p_t = cload("s01p_t", s01p, (D_STATE, 128))
        idb_t = cload("idb_t", ident_bf, (128, 128))
        idf_t = cload("idf_t", ident_f32, (128, 128), F32)
        ones_t = cload("ones_t", ones_bf, (128, 1))
        onesr_t = cload("onesr_t", ones_row_bf, (1, 128))
        msk0_t = cload("msk0_t", msk0, (XD, 1), F32)
        msk1_t = cload("msk1_t", msk1, (XD, 1), F32)
        eps_t = cpool.tile([128, 1], F32)
        nc.vector.memset(eps_t[:], EPS)
        r01_t = cpool.tile([128, 16 * 128], BF16)
        for k in range(16):
            nc.sync.dma_start(r01_t[:, 128 * k:128 * (k + 1)], r01[128 * k:128 * (k + 1), :])
        g01_t = cpool.tile([128, 16 * 128], BF16)
        for k in range(16):
            nc.sync.dma_start(g01_t[:, 128 * k:128 * (k + 1)], g01[128 * k:128 * (k + 1), :])

        # long-lived activations through the scan phase (freed before MLP)
        mid = top.enter_context(ExitStack())
        acts = mid.enter_context(tc.tile_pool(name="acts", bufs=1))
        u_bf = [acts.tile([128, T], BF16, tag=f"u{k}", name=f"u{k}") for k in range(NET)]
        sz_bf = [acts.tile([128, T], BF16, tag=f"sz{k}", name=f"sz{k}") for k in range(NET)]
        lnsig_bf = [acts.tile([128, T], BF16, tag=f"lns{k}", name=f"lns{k}") for k in range(NET)]
        brep_t = acts.tile([128, T], BF16, tag="brep", name="brep")
        crep_t = acts.tile([128, T], BF16, tag="crep", name="crep")
        dtlow_bf = acts.tile([DT_RANK, T], BF16, tag="dtlow", name="dtlow")

        # ============ P1+P2: rmsnorm, transpose, in_proj ============
        with ExitStack() as ph:
            winp = ph.enter_context(tc.tile_pool(name="win", bufs=1))
            w_in_t = [winp.tile([128, 2 * EL], BF16, tag=f"wi{k}", name=f"wi{k}") for k in range(8)]
            for k in range(8):
                nc.sync.dma_start(w_in_t[k][:], w_in_T[128 * k:128 * (k + 1), :])

            xinp = ph.enter_context(tc.tile_pool(name="xinz", bufs=1))
            x_in = [xinp.tile([128, T], BF16, tag=f"xin{k}", name=f"xin{k}") for k in range(NET)]

            with ExitStack() as p1:
                xnp = p1.enter_context(tc.tile_pool(name="xnT", bufs=1))
                xn_T = [xnp.tile([128, T], BF16, tag=f"xnT{k}", name=f"xnT{k}") for k in range(8)]
                str_p = p1.enter_context(tc.tile_pool(name="p1s", bufs=3))
                xtmp = p1.enter_context(tc.tile_pool(name="p1x", bufs=6))
                jp = p1.enter_context(tc.tile_pool(name="p1j", bufs=1))
                junk = jp.tile([128, D_MODEL], BF16)
                pps1 = p1.enter_context(tc.tile_pool(name="p1ps", bufs=2, space="PSUM"))

                for c in range(NCH):
                    xnt4 = []
                    for q in range(4):
                        i = 4 * c + q
                        xt = str_p.tile([128, D_MODEL], BF16, tag="xt", name="xt")
                        nc.gpsimd.dma_start(xt[:], xb[128 * i:128 * (i + 1), :])
                        ssq = str_p.tile([128, 1], F32, tag="ssq", name="ssq")
                        nc.scalar.activation(junk[:], xt[:], AF.Square, accum_out=ssq[:])
                        rr = str_p.tile([128, 1], F32, tag="rr", name="rr")
                        nc.scalar.activation(rr[:], ssq[:], AF.Sqrt, scale=1.0 / D_MODEL,
                                             bias=eps_t[:, 0:1])
                        rc = str_p.tile([128, 1], F32, tag="rc", name="rc")
                        nc.vector.reciprocal(rc[:], rr[:])
                        xnt = xtmp.tile([128, D_MODEL], BF16, tag="xnt", name="xnt")
                        nc.vector.tensor_scalar_mul(xnt[:], xt[:], rc[:, 0:1])
                        xnt4.append(xnt)
                    for k in range(8):
                        pt = pps1.tile([128, 512], BF16, tag="pt", name="pt")
                        for q in range(4):
                            nc.tensor.transpose(pt[:, 128 * q:128 * (q + 1)],
                                                xnt4[q][:, 128 * k:128 * (k + 1)],
                                                idb_t[:])
                        nc.vector.tensor_copy(xn_T[k][:, 512 * c:512 * (c + 1)], pt[:])

                # in_proj
                pps2 = p1.enter_context(tc.tile_pool(name="p2ps", bufs=4, space="PSUM"))
                for m in range(8):
                    for c in range(NCH):
                        ps = pps2.tile([128, 512], F32, tag="ps", name="ps")
                        for k in range(8):
                            nc.tensor.matmul(ps[:], w_in_t[k][:, 128 * m:128 * (m + 1)],
                                             xn_T[k][:, 512 * c:512 * (c + 1)],
                                             start=(k == 0), stop=(k == 7))
                        if m < 4:
                            nc.scalar.copy(x_in[m][:, 512 * c:512 * (c + 1)], ps[:])
                        else:
                            nc.scalar.activation(sz_bf[m - 4][:, 512 * c:512 * (c + 1)],
                                                 ps[:], AF.Silu)

            # ============ P3: conv + silu -> u ============
            with ExitStack() as p3:
                c3 = p3.enter_context(tc.tile_pool(name="p3", bufs=2))
                for k in range(NET):
                    xc = c3.tile([128, T], BF16, tag="xc", name="xc")
                    nc.vector.tensor_scalar(xc[:], x_in[k][:], convw_t[:, 4 * k + 3:4 * k + 4],
                                            convb_t[:, k:k + 1], ALU.mult, ALU.add)
                    for sh in range(1, 4):
                        nc.vector.scalar_tensor_tensor(
                            xc[:, sh:T], x_in[k][:, 0:T - sh],
                            convw_t[:, 4 * k + 3 - sh:4 * k + 4 - sh],
                            xc[:, sh:T], ALU.mult, ALU.add)
                    nc.scalar.activation(u_bf[k][:], xc[:], AF.Silu)

        # ============ P4: x_proj partial -> AllReduce8 -> dtlow/B_rep/C_rep ============
        with ExitStack() as p4:
            wxp = p4.enter_context(tc.tile_pool(name="wxp", bufs=1))
            w_xp_t = [wxp.tile([128, XD], BF16, tag=f"wxp{k}", name=f"wxp{k}") for k in range(NET)]
            for k in range(NET):
                nc.sync.dma_start(w_xp_t[k][:], w_xp_T[128 * k:128 * (k + 1), :])
            pps = p4.enter_context(tc.tile_pool(name="p4ps", bufs=2, space="PSUM"))
            sp = p4.enter_context(tc.tile_pool(name="p4s", bufs=2))
            big = p4.enter_context(tc.tile_pool(name="p4big", bufs=1))
            for c in range(NCH):
                ps = pps.tile([XD, 512], F32, tag="ps4", name="ps4")
                for k in range(NET):
                    nc.tensor.matmul(ps[:], w_xp_t[k][:], u_bf[k][:, 512 * c:512 * (c + 1)],
                                     start=(k == 0), stop=(k == NET - 1))
                t0 = sp.tile([XD, 512], BF16, tag="t0", name="t0")
                nc.vector.tensor_scalar_mul(t0[:], ps[:], msk0_t[:, 0:1])
                t1 = sp.tile([XD, 512], BF16, tag="t1", name="t1")
                nc.vector.tensor_scalar_mul(t1[:], ps[:], msk1_t[:, 0:1])
                nc.sync.dma_start(xdbl_in[0:XD, 512 * c:512 * (c + 1)], t0[:])
                nc.sync.dma_start(xdbl_in[XD:2 * XD, 512 * c:512 * (c + 1)], t1[:])
            if nocc or nocc_ar:
                nc.sync.dma_start(xdbl_out, xdbl_in)
            else:
                nc.gpsimd.collective_compute("AllReduce", ALU.add, replica_groups=g8,
                                             ins=[xdbl_in], outs=[xdbl_out])
            # fill the AllReduce window: transpose the residual token-quarter
            # (f32) now and stage it in DRAM for the MLP tail
            ppq = p4.enter_context(tc.tile_pool(name="p4q", bufs=2, space="PSUM"))
            spq = p4.enter_context(tc.tile_pool(name="p4qs", bufs=2))
            for i in range(TQ // 128):
                xt_ = spq.tile([128, D_MODEL], F32, tag="xq_tm", name="xq_tm")
                nc.sync.dma_start(xt_[:], xq[128 * i:128 * (i + 1), :])
                for h in range(2):
                    ptq = ppq.tile([128, 512], F32, tag="ptq", name="ptq")
                    for q in range(4):
                        k = 4 * h + q
                        nc.tensor.transpose(ptq[:, 128 * q:128 * (q + 1)],
                                            xt_[:, 128 * k:128 * (k + 1)], idf_t[:])
                    otq = spq.tile([128, 512], F32, tag="otq", name="otq")
                    nc.scalar.copy(otq[:], ptq[:])
                    for q in range(4):
                        k = 4 * h + q
                        nc.sync.dma_start(
                            xqT_dram[128 * k:128 * (k + 1), 128 * i:128 * (i + 1)],
                            otq[:, 128 * q:128 * (q + 1)])
            # batch-select each 32-aligned section separately (compute ops
            # cannot start at partition 80)
            def _sel(rows, nrows, out_dt, nm):
                a0 = big.tile([nrows, T], BF16, tag=nm + "a0", name=nm + "a0")
                nc.sync.dma_start(a0[:], xdbl_out[rows:rows + nrows, :])
                a1 = big.tile([nrows, T], BF16, tag=nm + "a1", name=nm + "a1")
                nc.sync.dma_start(a1[:], xdbl_out[XD + rows:XD + rows + nrows, :])
                nc.vector.tensor_scalar_mul(a0[:], a0[:], msk0_t[0:nrows, 0:1])
                o = big.tile([nrows, T], out_dt, tag=nm, name=nm)
                nc.vector.scalar_tensor_tensor(o[:], a1[:], msk1_t[0:nrows, 0:1],
                                               a0[:], ALU.mult, ALU.add)
                return o
            dl = _sel(0, DT_RANK, BF16, "dl")
            nc.vector.tensor_copy(dtlow_bf[:], dl[:])
            b_sb = _sel(DT_RANK, D_STATE, BF16, "b_sb")
            c_sb = _sel(DT_RANK + D_STATE, D_STATE, BF16, "c_sb")
            pps2 = p4.enter_context(tc.tile_pool(name="p4ps2", bufs=2, space="PSUM"))
            for c in range(NCH):
                pb = pps2.tile([128, 512], F32, tag="pb", name="pb")
                nc.tensor.matmul(pb[:], s01n_t[:], b_sb[:, 512 * c:512 * (c + 1)],
                                 start=True, stop=True)
                nc.vector.tensor_copy(brep_t[:, 512 * c:512 * (c + 1)], pb[:])
                pc = pps2.tile([128, 512], F32, tag="pc", name="pc")
                nc.tensor.matmul(pc[:], s01p_t[:], c_sb[:, 512 * c:512 * (c + 1)],
                                 start=True, stop=True)
                nc.vector.tensor_copy(crep_t[:, 512 * c:512 * (c + 1)], pc[:])

        # ============ P5: dt path ============
        with ExitStack() as p5:
            wdt = p5.enter_context(tc.tile_pool(name="wdt", bufs=1))
            w_dt_t = wdt.tile([DT_RANK, EL], BF16)
            nc.sync.dma_start(w_dt_t[:], w_dt_T)
            pps = p5.enter_context(tc.tile_pool(name="p5ps", bufs=4, space="PSUM"))
            sp = p5.enter_context(tc.tile_pool(name="p5s", bufs=2))
            for m in range(NET):
                sg = sp.tile([128, T], F32, tag="sg", name="sg")
                for c in range(NCH):
                    ps = pps.tile([128, 512], F32, tag="ps5", name="ps5")
                    nc.tensor.matmul(ps[:], w_dt_t[:, 128 * m:128 * (m + 1)],
                                     dtlow_bf[:, 512 * c:512 * (c + 1)], start=True, stop=True)
                    nc.scalar.activation(sg[:, 512 * c:512 * (c + 1)], ps[:], AF.Sigmoid,
                                         scale=-1.0, bias=dtbn_t[:, m:m + 1])
                lns = sp.tile([128, T], F32, tag="lns", name="lns")
                nc.scalar.activation(lns[:], sg[:], AF.Ln)
                nc.vector.tensor_copy(lnsig_bf[m][:], lns[:])
                dtu = sp.tile([128, T], BF16, tag="dtu", name="dtu")
                nc.vector.tensor_tensor(dtu[:], lns[:], u_bf[m][:], ALU.mult)
                nc.sync.dma_start(dtu_dram[128 * m:128 * (m + 1), :], dtu[:])

        # ============ P6: scan + y-sum + gate ============
        y2p = mid.enter_context(tc.tile_pool(name="y2p", bufs=1))
        y2_bf = [y2p.tile([128, T], BF16, tag=f"y2{k}", name=f"y2{k}") for k in range(NET)]
        with ExitStack() as p6:
            reps = p6.enter_context(tc.tile_pool(name="reps", bufs=2, space="PSUM"))
            yps = p6.enter_context(tc.tile_pool(name="ypsum", bufs=1, space="PSUM"))
            sp = p6.enter_context(tc.tile_pool(name="p6s", bufs=3))
            for J in range(4):
                py = yps.tile([128, T], F32, tag="py", name="py")
                for jj in range(16):
                    j = 16 * J + jj
                    dA = sp.tile([128, T], F32, tag="dA", name="dA")
                    for hf in range(2):
                        pr = reps.tile([128, 1024], F32, tag="pr", name="pr")
                        for q in range(2):
                            c = 2 * hf + q
                            nc.tensor.matmul(pr[:, 512 * q:512 * (q + 1)],
                                             r01_t[:, 128 * jj:128 * (jj + 1)],
                                             lnsig_bf[J][:, 512 * c:512 * (c + 1)],
                                             start=True, stop=True)
                        nc.scalar.activation(dA[:, 1024 * hf:1024 * (hf + 1)], pr[:],
                                             AF.Exp, scale=negA_t[:, j:j + 1])
                    dtur = sp.tile([128, T], BF16, tag="dtur", name="dtur")
                    src = dtu_dram[128 * J + 8 * jj:128 * J + 8 * jj + 8, :]
                    nc.sync.dma_start(dtur[:], src.unsqueeze(0).broadcast_to([16, 8, T]))
                    # scan is DVE-only (walrus rejects it on Pool); the two
                    # elementwise multiplies can run on GpSimd to unload DVE
                    bb = sp.tile([128, T], BF16, tag="bb", name="bb")
                    nc.vector.tensor_tensor(bb[:], dtur[:], brep_t[:], ALU.mult)
                    hh = sp.tile([128, T], BF16, tag="hh", name="hh")
                    nc.vector.tensor_tensor_scan(hh[:], dA[:], bb[:], 0.0, ALU.mult, ALU.add)
                    # C-multiply is off the scan-to-scan chain: mostly GpSimd,
                    # 1/3 back on DVE so Pool doesn't become the phase bound
                    ve = (nc.gpsimd if (j % 4) else nc.vector) if gps_mod else nc.vector
                    t1 = sp.tile([128, T], BF16, tag="t1", name="t1")
                    ve.tensor_tensor(t1[:], hh[:], crep_t[:], ALU.mult)
                    for c in range(NCH):
                        nc.tensor.matmul(py[:, 512 * c:512 * (c + 1)],
                                         g01_t[:, 128 * jj:128 * (jj + 1)],
                                         t1[:, 512 * c:512 * (c + 1)],
                                         start=(jj == 0), stop=(jj == 15))
                for c in range(NCH):
                    yd = sp.tile([128, 512], F32, tag="yd", name="yd")
                    nc.vector.scalar_tensor_tensor(yd[:], u_bf[J][:, 512 * c:512 * (c + 1)],
                                                   dcol_t[:, J:J + 1],
                                                   py[:, 512 * c:512 * (c + 1)],
                                                   ALU.mult, ALU.add)
                    nc.vector.tensor_tensor(y2_bf[J][:, 512 * c:512 * (c + 1)], yd[:],
                                            sz_bf[J][:, 512 * c:512 * (c + 1)], ALU.mult)

        # ============ P7: out_proj partial -> ReduceScatter4 ============
        with ExitStack() as p7:
            wout = p7.enter_context(tc.tile_pool(name="wout", bufs=1))
            w_out_t = [wout.tile([128, D_MODEL], BF16, tag=f"wo{k}", name=f"wo{k}") for k in range(NET)]
            for k in range(NET):
                nc.sync.dma_start(w_out_t[k][:], w_out_T[128 * k:128 * (k + 1), :])
            pps = p7.enter_context(tc.tile_pool(name="p7ps", bufs=4, space="PSUM"))
            sp = p7.enter_context(tc.tile_pool(name="p7s", bufs=4))
            for m in range(8):
                for c in range(NCH):
                    ps = pps.tile([128, 512], F32, tag="ps7", name="ps7")
                    for k in range(NET):
                        nc.tensor.matmul(ps[:], w_out_t[k][:, 128 * m:128 * (m + 1)],
                                         y2_bf[k][:, 512 * c:512 * (c + 1)],
                                         start=(k == 0), stop=(k == NET - 1))
                    ob = sp.tile([128, 512], BF16, tag="ob", name="ob")
                    nc.scalar.copy(ob[:], ps[:])
                    rr = slice(D_MODEL * c + 128 * m, D_MODEL * c + 128 * (m + 1))
                    nc.sync.dma_start(rs_in_h[0][rr, :], ob[:, 0:TQ // 2])
                    nc.sync.dma_start(rs_in_h[1][rr, :], ob[:, TQ // 2:TQ])
            for h in range(2):
                if nocc or nocc_rs:
                    nc.sync.dma_start(rs_out_h[h], rs_in_h[h][0:D_MODEL, :])
                else:
                    nc.gpsimd.collective_compute("ReduceScatter", ALU.add,
                                                 replica_groups=g4,
                                                 ins=[rs_in_h[h]], outs=[rs_out_h[h]])

        mid.close()

        # ============ P8: MLP tail ============
        with ExitStack() as p8:
            wmlp = p8.enter_context(tc.tile_pool(name="wmlp", bufs=1))
            w_fc_t = [wmlp.tile([128, 2 * D_MODEL], BF16, tag=f"wf{k}", name=f"wf{k}") for k in range(8)]
            for k in range(8):
                nc.sync.dma_start(w_fc_t[k][:], w_fc_T[128 * k:128 * (k + 1), :])
            w_pr_t = [wmlp.tile([128, D_MODEL], BF16, tag=f"wp{k}", name=f"wp{k}") for k in range(16)]
            for k in range(16):
                nc.sync.dma_start(w_pr_t[k][:], w_pr_T[128 * k:128 * (k + 1), :])

            ar = p8.enter_context(tc.tile_pool(name="p8a", bufs=1))
            st = p8.enter_context(tc.tile_pool(name="p8t", bufs=2))
            ppt = p8.enter_context(tc.tile_pool(name="p8pt", bufs=2, space="PSUM"))
            ppm = p8.enter_context(tc.tile_pool(name="p8pm", bufs=2, space="PSUM"))
            pp1 = p8.enter_context(tc.tile_pool(name="p8p1", bufs=1, space="PSUM"))

            TH = TQ // 2
            for th in range(2):
                t0 = TH * th
                x2_T = [ar.tile([128, TH], F32, tag=f"x2T{k}", name=f"x2T{k}")
                        for k in range(8)]
                for k in range(8):
                    nc.sync.dma_start(x2_T[k][:], xqT_dram[128 * k:128 * (k + 1), t0:t0 + TH])
                rsb = [ar.tile([128, TH], BF16, tag=f"rsb{k}", name=f"rsb{k}") for k in range(8)]
                for k in range(8):
                    nc.sync.dma_start(rsb[k][:], rs_out_h[th][128 * k:128 * (k + 1), :])
                    nc.vector.tensor_tensor(x2_T[k][:], x2_T[k][:], rsb[k][:], ALU.add)

                # rmsnorm over features via ones-matmul
                sq = [ar.tile([128, TH], BF16, tag=f"sq{k}", name=f"sq{k}") for k in range(8)]
                for k in range(8):
                    nc.scalar.activation(sq[k][:], x2_T[k][:], AF.Square)
                pss = pp1.tile([1, TH], F32, tag="pss", name="pss")
                for k in range(8):
                    nc.tensor.matmul(pss[:], ones_t[:], sq[k][:], start=(k == 0), stop=(k == 7))
                rrow = st.tile([1, TH], F32, tag="rrow", name="rrow")
                nc.scalar.activation(rrow[:], pss[:], AF.Sqrt, scale=1.0 / D_MODEL,
                                     bias=eps_t[0:1, 0:1])
                rrec = st.tile([1, TH], F32, tag="rrec", name="rrec")
                nc.vector.reciprocal(rrec[:], rrow[:])
                rbf = st.tile([1, TH], BF16, tag="rbf", name="rbf")
                nc.vector.tensor_copy(rbf[:], rrec[:])
                pr2 = pp1.tile([128, TH], F32, tag="pr2", name="pr2")
                nc.tensor.matmul(pr2[:], onesr_t[:], rbf[:], start=True, stop=True)
                x2n = [ar.tile([128, TH], BF16, tag=f"x2n{k}", name=f"x2n{k}") for k in range(8)]
                for k in range(8):
                    nc.vector.tensor_tensor(x2n[k][:], x2_T[k][:], pr2[:], ALU.mult)

                # c_fc + relu^2
                hh_t = [ar.tile([128, TH], BF16, tag=f"hh{k}", name=f"hh{k}") for k in range(16)]
                for m in range(16):
                    pm = ppm.tile([128, TH], F32, tag="pmm", name="pmm")
                    for k in range(8):
                        nc.tensor.matmul(pm[:], w_fc_t[k][:, 128 * m:128 * (m + 1)], x2n[k][:],
                                         start=(k == 0), stop=(k == 7))
                    rl = st.tile([128, TH], BF16, tag="rl", name="rl")
                    nc.scalar.activation(rl[:], pm[:], AF.Relu)
                    nc.vector.tensor_tensor(hh_t[m][:], rl[:], rl[:], ALU.mult)
                # c_proj + residual
                fin = [ar.tile([128, TH], F32, tag=f"fin{k}", name=f"fin{k}") for k in range(8)]
                for m in range(8):
                    pm = ppm.tile([128, TH], F32, tag="pmm", name="pmm")
                    for k in range(16):
                        nc.tensor.matmul(pm[:], w_pr_t[k][:, 128 * m:128 * (m + 1)], hh_t[k][:],
                                         start=(k == 0), stop=(k == 15))
                    nc.vector.tensor_tensor(fin[m][:], x2_T[m][:], pm[:], ALU.add)
                # transpose to token-major + store
                for i in range(TH // 128):
                    for h in range(2):
                        pt = ppt.tile([128, 512], F32, tag="ptx", name="ptx")
                        for q in range(4):
                            m = 4 * h + q
                            nc.tensor.transpose(pt[:, 128 * q:128 * (q + 1)],
                                                fin[m][:, 128 * i:128 * (i + 1)], idf_t[:])
                        ot = st.tile([128, 512], F32, tag="ot", name="ot")
                        nc.scalar.copy(ot[:], pt[:])
                        nc.sync.dma_start(out[t0 + 128 * i:t0 + 128 * (i + 1),
                                              512 * h:512 * (h + 1)], ot[:])

    nc.compile()
    return nc


def _prep_inputs(inputs):
    x = np.asarray(inputs['x'], np.float32)
    in_proj_w = np.asarray(inputs['in_proj_w'], np.float32)
    conv_w = np.asarray(inputs['conv_w'], np.float32)
    conv_b = np.asarray(inputs['conv_b'], np.float32)
    x_proj_w = np.asarray(inputs['x_proj_w'], np.float32)
    dt_proj_w = np.asarray(inputs['dt_proj_w'], np.float32)
    dt_proj_b = np.asarray(inputs['dt_proj_b'], np.float32)
    A_log = np.asarray(inputs['A_log'], np.float32)
    D = np.asarray(inputs['D'], np.float32)
    out_proj_w = np.asarray(inputs['out_proj_w'], np.float32)
    c_fc_w = np.asarray(inputs['c_fc_w'], np.float32)
    c_proj_w = np.asarray(inputs['c_proj_w'], np.float32)

    import ml_dtypes
    bf = lambda a: np.ascontiguousarray(a).astype(ml_dtypes.bfloat16)
    f32 = lambda a: np.ascontiguousarray(a, np.float32)

    r01 = np.zeros((16, 128, 128), np.float32)  # [jm][k, m] = 1 iff k == 8*jm + m%8
    g01 = np.zeros((16, 128, 128), np.float32)  # [jm][k, m] = 1 iff m == 8*jm + k%8
    for jm in range(16):
        for m in range(128):
            r01[jm, 8 * jm + (m % 8), m] = 1.0
            g01[jm, m, 8 * jm + (m % 8)] = 1.0
    s01n = np.zeros((D_STATE, 128), np.float32)
    s01p = np.zeros((D_STATE, 128), np.float32)
    for m in range(128):
        s01n[m // 8, m] = -1.0
        s01p[m // 8, m] = 1.0
    ident = np.eye(128, dtype=np.float32)

    def col_fold(a):
        # (EL,) or (EL, w) -> (128, NET*w): cols [w*k:w*(k+1)] = rows of e-tile k
        a = a.reshape(EL, -1)
        w = a.shape[1]
        o = np.zeros((128, NET * w), np.float32)
        for k in range(NET):
            o[:, w * k:w * (k + 1)] = a[128 * k:128 * (k + 1)]
        return o

    in_maps = []
    for c in range(8):
        b, r = c // 4, c % 4
        sl = slice(EL * r, EL * (r + 1))
        negA_ = np.zeros((128, NJ), np.float32)
        p = np.arange(128)
        for j in range(NJ):
            e = EL * r + 8 * j + (p % 8)
            s = p // 8
            negA_[:, j] = np.exp(A_log[e, s])
        msk0 = np.full((XD, 1), 1.0 if b == 0 else 0.0, np.float32)
        msk1 = np.full((XD, 1), 1.0 if b == 1 else 0.0, np.float32)
        in_maps.append({
            'xb': f32(x[b]),
            'xq': f32(x[b][TQ * r:TQ * (r + 1)]),
            'w_in_T': bf(np.concatenate([in_proj_w[sl], in_proj_w[D_INNER:][sl]], 0).T),
            'conv_wc': col_fold(conv_w[sl]),
            'conv_bc': col_fold(conv_b[sl]),
            'w_xp_T': bf(x_proj_w[:, sl].T),
            'w_dt_T': bf(dt_proj_w[sl].T),
            'dt_bnc': col_fold(-dt_proj_b[sl]),
            'negA': negA_,
            'd_c': col_fold(D[sl]),
            'w_out_T': bf(out_proj_w[:, sl].T),
            'w_fc_T': bf(c_fc_w.T),
            'w_pr_T': bf(c_proj_w.T),
            'r01': bf(r01.reshape(16 * 128, 128)),
            'g01': bf(g01.reshape(16 * 128, 128)),
            's01n': bf(s01n),
            's01p': bf(s01p),
            'ident_bf': bf(ident),
            'ident_f32': f32(ident),
            'ones_bf': bf(np.ones((128, 1), np.float32)),
            'ones_row_bf': bf(np.ones((1, 128), np.float32)),
            'msk0': msk0,
            'msk1': msk1,
        })
    return in_maps


def kernel(**inputs) -> np.ndarray:
    if 'nc' not in _CACHE:
        _CACHE['nc'] = _build()
    nc = _CACHE['nc']
    in_maps = _prep_inputs(inputs)
    res = run_bass_kernel_spmd(nc, in_maps, core_ids=list(range(8)))
    out = np.zeros((B, T, D_MODEL), np.float32)
    for c in range(8):
        b, r = c // 4, c % 4
        out[b, TQ * r:TQ * (r + 1), :] = res.results[c]['out']
    return out



# revision 11
# speedup vs baseline: 1.2019x; 1.2019x over previous
"""Trainium2 Bass kernel for nn_Block_4526895530469 (Mamba block + MLP residual).

Sharding over 8 NeuronCores: core c -> batch b=c//4, channel shard r=c%4
(512 of the 2048 d_inner channels), full T=2048 sequence per core. The
selective scan runs full-T per channel on the Vector engine
(tensor_tensor_scan), so there is no cross-core state chain. Layout for the
scan is (s,e)-pairs on partitions (row p = 8*s + e_local, 16 states x 8
channels per 128-row tile) x time on the free dimension.

Collectives: one zero-padded global AllReduce for the (96, T) x_dbl partial
sums (contraction over the sharded d_inner), and one 4-group ReduceScatter
for the out_proj partials which simultaneously scatters tokens for the
token-parallel MLP tail.
"""
import sys
sys.path.insert(0, '/opt/trn_rl_repo')

import numpy as np
from contextlib import ExitStack

import concourse.bass as bass
from concourse import bacc
import concourse.tile as tile
from concourse import mybir
from concourse.bass_utils import run_bass_kernel_spmd

# The interp (used by Tile's scheduling pass and by test simulation) lacks
# Silu; emulate it: run the existing Sigmoid path, then multiply by the
# scaled/biased input.
from concourse import bass_interp as _bi
from concourse import mybir as _mb

_orig_visit_act = _bi.InstructionExecutor.visit_InstActivation


def _visit_act_with_silu(self, instruction, *a, **kw):
    _extra = (_mb.ActivationFunctionType.Silu, _mb.ActivationFunctionType.Softplus)
    if instruction.func not in _extra:
        return _orig_visit_act(self, instruction, *a, **kw)
    import numpy as _np
    assert len(instruction.outs) == 1, "act shim: no accum_out support"
    func0 = instruction.func
    try:
        instruction.func = _mb.ActivationFunctionType.Sigmoid
        res = _orig_visit_act(self, instruction, *a, **kw)
    finally:
        instruction.func = func0
    reg_snapshot = kw.get("reg_snapshot")
    inp = self.view_ap(instruction.ins[0], _bi.Direction.READ, instruction,
                       reg_snapshot=reg_snapshot).astype(_np.float32)
    inp = inp.reshape(inp.shape[0], -1)

    def _val(arg):
        if isinstance(arg, _mb.ImmediateValue):
            return arg.value
        v = self.view_ap(arg, _bi.Direction.READ, instruction,
                         reg_snapshot=reg_snapshot).astype(_np.float32)
        return v.reshape(v.shape[0], -1)

    bias = _val(instruction.ins[1])
    scale = _val(instruction.ins[2])
    sx = inp * scale + bias
    out_view = self.view_ap(instruction.outs[0], _bi.Direction.WRITE, instruction,
                            reg_snapshot=reg_snapshot)
    if func0 == _mb.ActivationFunctionType.Silu:
        sig = _np.asarray(out_view, dtype=_np.float32).reshape(sx.shape)
        val = sig * sx
    else:  # Softplus
        val = _np.logaddexp(0.0, sx)
    out_view[:] = val.reshape(out_view.shape).astype(out_view.dtype)
    return res


_bi.InstructionExecutor.visit_InstActivation = _visit_act_with_silu

F32 = mybir.dt.float32
BF16 = mybir.dt.bfloat16
AF = mybir.ActivationFunctionType
ALU = mybir.AluOpType

D_MODEL, D_INNER, D_STATE, D_CONV, DT_RANK = 1024, 2048, 16, 4, 64
B, T = 2, 2048
EL = D_INNER // 4          # 512 channels per core
NET = EL // 128            # 4 e-tiles
NJ = EL // 8               # 64 scan tiles
NCH = T // 512             # 4 t-chunks
TQ = T // 4                # 512 tokens for the MLP tail
XD = DT_RANK + 2 * D_STATE  # 96
EPS = float(np.finfo(np.float32).eps)

_CACHE = {}


def _build(nocc=False, gps_mod=0, ar_bf16=True, nocc_ar=False, nocc_rs=False, rs_split=True):
    nc = bacc.Bacc("TRN2", target_bir_lowering=False, debug=False, num_devices=8)

    def din(name, shape, dt=BF16):
        return nc.dram_tensor(name, list(shape), dt, kind="ExternalInput").ap()

    xb = din("xb", (T, D_MODEL), F32)
    xq = din("xq", (TQ, D_MODEL), F32)
    w_in_T = din("w_in_T", (D_MODEL, 2 * EL))
    conv_wc = din("conv_wc", (128, NET * D_CONV), F32)   # cols [4k:4k+4] = e-tile k
    conv_bc = din("conv_bc", (128, NET), F32)
    w_xp_T = din("w_xp_T", (EL, XD))
    w_dt_T = din("w_dt_T", (DT_RANK, EL))
    dt_bnc = din("dt_bnc", (128, NET), F32)              # +dt_proj_b
    negA = din("negA", (128, NJ), F32)                   # A = -exp(A_log) per scan tile col
    d_c = din("d_c", (128, NET), F32)
    d_diag = din("d_diag", (128, NET * 128))             # diag(D) per e-tile
    w_out_T = din("w_out_T", (EL, D_MODEL))
    w_fc_T = din("w_fc_T", (D_MODEL, 2 * D_MODEL))
    w_pr_T = din("w_pr_T", (2 * D_MODEL, D_MODEL))
    r01 = din("r01", (16 * 128, 128))
    g01 = din("g01", (16 * 128, 128))
    s01n = din("s01n", (D_STATE, 128))
    s01p = din("s01p", (D_STATE, 128))
    ident_bf = din("ident_bf", (128, 128))
    ident_f32 = din("ident_f32", (128, 128), F32)
    ones_bf = din("ones_bf", (128, 1))
    ones_row_bf = din("ones_row_bf", (1, 128))
    msk0 = din("msk0", (XD, 1), F32)
    msk1 = din("msk1", (XD, 1), F32)

    out = nc.dram_tensor("out", [TQ, D_MODEL], F32, kind="ExternalOutput").ap()

    xdbl_in = nc.dram_tensor("xdbl_in", [2 * XD, T], BF16).ap()
    xdbl_out = nc.dram_tensor("xdbl_out", [2 * XD, T], BF16).ap()
    rs_in_h = [nc.dram_tensor(f"rs_in{h}", [4 * D_MODEL, TQ // 2], BF16).ap() for h in range(2)]
    rs_out_h = [nc.dram_tensor(f"rs_out{h}", [D_MODEL, TQ // 2], BF16).ap() for h in range(2)]
    dtu_dram = nc.dram_tensor("dtu_dram", [EL, T], BF16).ap()
    xqT_dram = nc.dram_tensor("xqT_dram", [D_MODEL, TQ], F32).ap()

    g8 = [[0, 1, 2, 3, 4, 5, 6, 7]]
    g4 = [[0, 1, 2, 3], [4, 5, 6, 7]]

    with tile.TileContext(nc) as tc, ExitStack() as top:
        cpool = top.enter_context(tc.tile_pool(name="consts", bufs=1))

        def cload(nm, name_ap, shape, dt=BF16):
            t = cpool.tile(list(shape), dt, tag=nm, name=nm)
            nc.sync.dma_start(t[:], name_ap)
            return t

        negA_t = cload("negA_t", negA, (128, NJ), F32)
        convw_t = cload("convw_t", conv_wc, (128, NET * D_CONV), F32)
        convb_t = cload("convb_t", conv_bc, (128, NET), F32)
        dtbn_t = cload("dtbn_t", dt_bnc, (128, NET), F32)
        dcol_t = cload("dcol_t", d_c, (128, NET), F32)
        ddiag_t = cload("ddiag_t", d_diag, (128, NET * 128))
        s01n_t = cload("s01n_t", s01n, (D_STATE, 128))
        s01p_t = cload("s01p_t", s01p, (D_STATE, 128))
        idb_t = cload("idb_t", ident_bf, (128, 128))
        idf_t = cload("idf_t", ident_f32, (128, 128), F32)
        ones_t = cload("ones_t", ones_bf, (128, 1))
        onesr_t = cload("onesr_t", ones_row_bf, (1, 128))
        msk0_t = cload("msk0_t", msk0, (XD, 1), F32)
        msk1_t = cload("msk1_t", msk1, (XD, 1), F32)
        eps_t = cpool.tile([128, 1], F32)
        nc.vector.memset(eps_t[:], EPS)
        r01_t = cpool.tile([128, 16 * 128], BF16)
        for k in range(16):
            nc.sync.dma_start(r01_t[:, 128 * k:128 * (k + 1)], r01[128 * k:128 * (k + 1), :])
        g01_t = cpool.tile([128, 16 * 128], BF16)
        for k in range(16):
            nc.sync.dma_start(g01_t[:, 128 * k:128 * (k + 1)], g01[128 * k:128 * (k + 1), :])

        # long-lived activations through the scan phase (freed before MLP)
        mid = top.enter_context(ExitStack())
        acts = mid.enter_context(tc.tile_pool(name="acts", bufs=1))
        u_bf = [acts.tile([128, T], BF16, tag=f"u{k}", name=f"u{k}") for k in range(NET)]
        sz_bf = [acts.tile([128, T], BF16, tag=f"sz{k}", name=f"sz{k}") for k in range(NET)]
        lnsig_bf = [acts.tile([128, T], BF16, tag=f"lns{k}", name=f"lns{k}") for k in range(NET)]
        brep_t = acts.tile([128, T], BF16, tag="brep", name="brep")
        crep_t = acts.tile([128, T], BF16, tag="crep", name="crep")
        dtlow_bf = acts.tile([DT_RANK, T], BF16, tag="dtlow", name="dtlow")

        # ============ P1+P2: rmsnorm, transpose, in_proj ============
        with ExitStack() as ph:
            winp = ph.enter_context(tc.tile_pool(name="win", bufs=1))
            w_in_t = [winp.tile([128, 2 * EL], BF16, tag=f"wi{k}", name=f"wi{k}") for k in range(8)]
            for k in range(8):
                nc.sync.dma_start(w_in_t[k][:], w_in_T[128 * k:128 * (k + 1), :])

            xinp = ph.enter_context(tc.tile_pool(name="xinz", bufs=1))
            x_in = [xinp.tile([128, T], BF16, tag=f"xin{k}", name=f"xin{k}") for k in range(NET)]

            with ExitStack() as p1:
                xnp = p1.enter_context(tc.tile_pool(name="xnT", bufs=1))
                xn_T = [xnp.tile([128, T], BF16, tag=f"xnT{k}", name=f"xnT{k}") for k in range(8)]
                str_p = p1.enter_context(tc.tile_pool(name="p1s", bufs=3))
                xtmp = p1.enter_context(tc.tile_pool(name="p1x", bufs=6))
                jp = p1.enter_context(tc.tile_pool(name="p1j", bufs=1))
                junk = jp.tile([128, D_MODEL], BF16)
                pps1 = p1.enter_context(tc.tile_pool(name="p1ps", bufs=2, space="PSUM"))

                for c in range(NCH):
                    xnt4 = []
                    for q in range(4):
                        i = 4 * c + q
                        xt = str_p.tile([128, D_MODEL], BF16, tag="xt", name="xt")
                        nc.gpsimd.dma_start(xt[:], xb[128 * i:128 * (i + 1), :])
                        ssq = str_p.tile([128, 1], F32, tag="ssq", name="ssq")
                        nc.scalar.activation(junk[:], xt[:], AF.Square, accum_out=ssq[:])
                        rr = str_p.tile([128, 1], F32, tag="rr", name="rr")
                        nc.scalar.activation(rr[:], ssq[:], AF.Sqrt, scale=1.0 / D_MODEL,
                                             bias=eps_t[:, 0:1])
                        rc = str_p.tile([128, 1], F32, tag="rc", name="rc")
                        nc.vector.reciprocal(rc[:], rr[:])
                        xnt = xtmp.tile([128, D_MODEL], BF16, tag="xnt", name="xnt")
                        nc.vector.tensor_scalar_mul(xnt[:], xt[:], rc[:, 0:1])
                        xnt4.append(xnt)
                    for k in range(8):
                        pt = pps1.tile([128, 512], BF16, tag="pt", name="pt")
                        for q in range(4):
                            nc.tensor.transpose(pt[:, 128 * q:128 * (q + 1)],
                                                xnt4[q][:, 128 * k:128 * (k + 1)],
                                                idb_t[:])
                        nc.vector.tensor_copy(xn_T[k][:, 512 * c:512 * (c + 1)], pt[:])

                # in_proj
                pps2 = p1.enter_context(tc.tile_pool(name="p2ps", bufs=4, space="PSUM"))
                for m in range(8):
                    for c in range(NCH):
                        ps = pps2.tile([128, 512], F32, tag="ps", name="ps")
                        for k in range(8):
                            nc.tensor.matmul(ps[:], w_in_t[k][:, 128 * m:128 * (m + 1)],
                                             xn_T[k][:, 512 * c:512 * (c + 1)],
                                             start=(k == 0), stop=(k == 7))
                        if m < 4:
                            nc.scalar.copy(x_in[m][:, 512 * c:512 * (c + 1)], ps[:])
                        else:
                            nc.scalar.activation(sz_bf[m - 4][:, 512 * c:512 * (c + 1)],
                                                 ps[:], AF.Silu)

            # ============ P3: conv + silu -> u ============
            with ExitStack() as p3:
                c3 = p3.enter_context(tc.tile_pool(name="p3", bufs=2))
                for k in range(NET):
                    xc = c3.tile([128, T], BF16, tag="xc", name="xc")
                    nc.vector.tensor_scalar(xc[:], x_in[k][:], convw_t[:, 4 * k + 3:4 * k + 4],
                                            convb_t[:, k:k + 1], ALU.mult, ALU.add)
                    for sh in range(1, 4):
                        nc.vector.scalar_tensor_tensor(
                            xc[:, sh:T], x_in[k][:, 0:T - sh],
                            convw_t[:, 4 * k + 3 - sh:4 * k + 4 - sh],
                            xc[:, sh:T], ALU.mult, ALU.add)
                    nc.scalar.activation(u_bf[k][:], xc[:], AF.Silu)

        # ============ P4: x_proj partial -> AllReduce8 -> dtlow/B_rep/C_rep ============
        with ExitStack() as p4:
            wxp = p4.enter_context(tc.tile_pool(name="wxp", bufs=1))
            w_xp_t = [wxp.tile([128, XD], BF16, tag=f"wxp{k}", name=f"wxp{k}") for k in range(NET)]
            for k in range(NET):
                nc.sync.dma_start(w_xp_t[k][:], w_xp_T[128 * k:128 * (k + 1), :])
            pps = p4.enter_context(tc.tile_pool(name="p4ps", bufs=2, space="PSUM"))
            sp = p4.enter_context(tc.tile_pool(name="p4s", bufs=2))
            big = p4.enter_context(tc.tile_pool(name="p4big", bufs=1))
            for c in range(NCH):
                ps = pps.tile([XD, 512], F32, tag="ps4", name="ps4")
                for k in range(NET):
                    nc.tensor.matmul(ps[:], w_xp_t[k][:], u_bf[k][:, 512 * c:512 * (c + 1)],
                                     start=(k == 0), stop=(k == NET - 1))
                t0 = sp.tile([XD, 512], BF16, tag="t0", name="t0")
                nc.vector.tensor_scalar_mul(t0[:], ps[:], msk0_t[:, 0:1])
                t1 = sp.tile([XD, 512], BF16, tag="t1", name="t1")
                nc.vector.tensor_scalar_mul(t1[:], ps[:], msk1_t[:, 0:1])
                nc.sync.dma_start(xdbl_in[0:XD, 512 * c:512 * (c + 1)], t0[:])
                nc.sync.dma_start(xdbl_in[XD:2 * XD, 512 * c:512 * (c + 1)], t1[:])
            if nocc or nocc_ar:
                nc.sync.dma_start(xdbl_out, xdbl_in)
            else:
                nc.gpsimd.collective_compute("AllReduce", ALU.add, replica_groups=g8,
                                             ins=[xdbl_in], outs=[xdbl_out])
            # fill the AllReduce window: transpose the residual token-quarter
            # (f32) now and stage it in DRAM for the MLP tail
            ppq = p4.enter_context(tc.tile_pool(name="p4q", bufs=2, space="PSUM"))
            spq = p4.enter_context(tc.tile_pool(name="p4qs", bufs=2))
            for i in range(TQ // 128):
                xt_ = spq.tile([128, D_MODEL], F32, tag="xq_tm", name="xq_tm")
                nc.sync.dma_start(xt_[:], xq[128 * i:128 * (i + 1), :])
                for h in range(2):
                    ptq = ppq.tile([128, 512], F32, tag="ptq", name="ptq")
                    for q in range(4):
                        k = 4 * h + q
                        nc.tensor.transpose(ptq[:, 128 * q:128 * (q + 1)],
                                            xt_[:, 128 * k:128 * (k + 1)], idf_t[:])
                    otq = spq.tile([128, 512], F32, tag="otq", name="otq")
                    nc.scalar.copy(otq[:], ptq[:])
                    for q in range(4):
                        k = 4 * h + q
                        nc.sync.dma_start(
                            xqT_dram[128 * k:128 * (k + 1), 128 * i:128 * (i + 1)],
                            otq[:, 128 * q:128 * (q + 1)])
            # batch-select each 32-aligned section separately (compute ops
            # cannot start at partition 80)
            def _sel(rows, nrows, out_dt, nm):
                a0 = big.tile([nrows, T], BF16, tag=nm + "a0", name=nm + "a0")
                nc.sync.dma_start(a0[:], xdbl_out[rows:rows + nrows, :])
                a1 = big.tile([nrows, T], BF16, tag=nm + "a1", name=nm + "a1")
                nc.sync.dma_start(a1[:], xdbl_out[XD + rows:XD + rows + nrows, :])
                nc.vector.tensor_scalar_mul(a0[:], a0[:], msk0_t[0:nrows, 0:1])
                o = big.tile([nrows, T], out_dt, tag=nm, name=nm)
                nc.vector.scalar_tensor_tensor(o[:], a1[:], msk1_t[0:nrows, 0:1],
                                               a0[:], ALU.mult, ALU.add)
                return o
            dl = _sel(0, DT_RANK, BF16, "dl")
            nc.vector.tensor_copy(dtlow_bf[:], dl[:])
            b_sb = _sel(DT_RANK, D_STATE, BF16, "b_sb")
            c_sb = _sel(DT_RANK + D_STATE, D_STATE, BF16, "c_sb")
            pps2 = p4.enter_context(tc.tile_pool(name="p4ps2", bufs=2, space="PSUM"))
            for c in range(NCH):
                pb = pps2.tile([128, 512], F32, tag="pb", name="pb")
                nc.tensor.matmul(pb[:], s01p_t[:], b_sb[:, 512 * c:512 * (c + 1)],
                                 start=True, stop=True)
                nc.vector.tensor_copy(brep_t[:, 512 * c:512 * (c + 1)], pb[:])
                pc = pps2.tile([128, 512], F32, tag="pc", name="pc")
                nc.tensor.matmul(pc[:], s01p_t[:], c_sb[:, 512 * c:512 * (c + 1)],
                                 start=True, stop=True)
                nc.vector.tensor_copy(crep_t[:, 512 * c:512 * (c + 1)], pc[:])

        # ============ P5: dt path ============
        with ExitStack() as p5:
            wdt = p5.enter_context(tc.tile_pool(name="wdt", bufs=1))
            w_dt_t = wdt.tile([DT_RANK, EL], BF16)
            nc.sync.dma_start(w_dt_t[:], w_dt_T)
            pps = p5.enter_context(tc.tile_pool(name="p5ps", bufs=4, space="PSUM"))
            sp = p5.enter_context(tc.tile_pool(name="p5s", bufs=2))
            for m in range(NET):
                for c in range(NCH):
                    ps = pps.tile([128, 512], F32, tag="ps5", name="ps5")
                    nc.tensor.matmul(ps[:], w_dt_t[:, 128 * m:128 * (m + 1)],
                                     dtlow_bf[:, 512 * c:512 * (c + 1)], start=True, stop=True)
                    # dt = softplus(ps + b) = ln(1 + e^(ps+b)); exp and ln share
                    # one ACT table set (natural_log_exp_and_others), also shared
                    # with the scan-phase exps -> no table thrash.
                    e1 = sp.tile([128, 512], F32, tag="e1", name="e1")
                    nc.scalar.activation(e1[:], ps[:], AF.Exp, bias=dtbn_t[:, m:m + 1])
                    nc.scalar.activation(lnsig_bf[m][:, 512 * c:512 * (c + 1)], e1[:],
                                         AF.Ln, bias=1.0)
                dtu = sp.tile([128, T], BF16, tag="dtu", name="dtu")
                nc.vector.tensor_tensor(dtu[:], lnsig_bf[m][:], u_bf[m][:], ALU.mult)
                nc.sync.dma_start(dtu_dram[128 * m:128 * (m + 1), :], dtu[:])

        # ============ P6: scan + y-sum + gate ============
        y2p = mid.enter_context(tc.tile_pool(name="y2p", bufs=1))
        y2_bf = [y2p.tile([128, T], BF16, tag=f"y2{k}", name=f"y2{k}") for k in range(NET)]
        with ExitStack() as p6:
            reps = p6.enter_context(tc.tile_pool(name="reps", bufs=2, space="PSUM"))
            yps = p6.enter_context(tc.tile_pool(name="ypsum", bufs=1, space="PSUM"))
            sp = p6.enter_context(tc.tile_pool(name="p6s", bufs=3))
            for J in range(4):
                py = yps.tile([128, T], F32, tag="py", name="py")
                for jj in range(16):
                    j = 16 * J + jj
                    dA = sp.tile([128, T], BF16, tag="dA", name="dA")
                    for hf in range(2):
                        pr = reps.tile([128, 1024], F32, tag="pr", name="pr")
                        for q in range(2):
                            c = 2 * hf + q
                            nc.tensor.matmul(pr[:, 512 * q:512 * (q + 1)],
                                             r01_t[:, 128 * jj:128 * (jj + 1)],
                                             lnsig_bf[J][:, 512 * c:512 * (c + 1)],
                                             start=True, stop=True)
                        nc.scalar.activation(dA[:, 1024 * hf:1024 * (hf + 1)], pr[:],
                                             AF.Exp, scale=negA_t[:, j:j + 1])
                    dtur = sp.tile([128, T], BF16, tag="dtur", name="dtur")
                    src = dtu_dram[128 * J + 8 * jj:128 * J + 8 * jj + 8, :]
                    nc.sync.dma_start(dtur[:], src.unsqueeze(0).broadcast_to([16, 8, T]))
                    # scan is DVE-only (walrus rejects it on Pool); GpSimd
                    # elementwise is avoided entirely: it locks the shared
                    # DVE/GpSimd SBUF port pair and slows DVE 2-3x.
                    bb = sp.tile([128, T], BF16, tag="bb", name="bb")
                    nc.vector.tensor_tensor(bb[:], dtur[:], brep_t[:], ALU.mult)
                    hh = sp.tile([128, T], BF16, tag="hh", name="hh")
                    nc.vector.tensor_tensor_scan(hh[:], dA[:], bb[:], 0.0, ALU.mult, ALU.add)
                    t1 = sp.tile([128, T], BF16, tag="t1", name="t1")
                    nc.vector.tensor_tensor(t1[:], hh[:], crep_t[:], ALU.mult)
                    for c in range(NCH):
                        nc.tensor.matmul(py[:, 512 * c:512 * (c + 1)],
                                         g01_t[:, 128 * jj:128 * (jj + 1)],
                                         t1[:, 512 * c:512 * (c + 1)],
                                         start=(jj == 0), stop=False)
                # u*D skip folded into the PSUM accumulation via diag(D)
                for c in range(NCH):
                    nc.tensor.matmul(py[:, 512 * c:512 * (c + 1)],
                                     ddiag_t[:, 128 * J:128 * (J + 1)],
                                     u_bf[J][:, 512 * c:512 * (c + 1)],
                                     start=False, stop=True)
                for c in range(NCH):
                    nc.vector.tensor_tensor(y2_bf[J][:, 512 * c:512 * (c + 1)],
                                            py[:, 512 * c:512 * (c + 1)],
                                            sz_bf[J][:, 512 * c:512 * (c + 1)], ALU.mult)

        # ============ P7: out_proj partial -> ReduceScatter4 ============
        with ExitStack() as p7:
            wout = p7.enter_context(tc.tile_pool(name="wout", bufs=1))
            w_out_t = [wout.tile([128, D_MODEL], BF16, tag=f"wo{k}", name=f"wo{k}") for k in range(NET)]
            for k in range(NET):
                nc.sync.dma_start(w_out_t[k][:], w_out_T[128 * k:128 * (k + 1), :])
            pps = p7.enter_context(tc.tile_pool(name="p7ps", bufs=4, space="PSUM"))
            sp = p7.enter_context(tc.tile_pool(name="p7s", bufs=4))
            for m in range(8):
                for c in range(NCH):
                    ps = pps.tile([128, 512], F32, tag="ps7", name="ps7")
                    for k in range(NET):
                        nc.tensor.matmul(ps[:], w_out_t[k][:, 128 * m:128 * (m + 1)],
                                         y2_bf[k][:, 512 * c:512 * (c + 1)],
                                         start=(k == 0), stop=(k == NET - 1))
                    ob = sp.tile([128, 512], BF16, tag="ob", name="ob")
                    nc.scalar.copy(ob[:], ps[:])
                    rr = slice(D_MODEL * c + 128 * m, D_MODEL * c + 128 * (m + 1))
                    nc.sync.dma_start(rs_in_h[0][rr, :], ob[:, 0:TQ // 2])
                    nc.sync.dma_start(rs_in_h[1][rr, :], ob[:, TQ // 2:TQ])
            for h in range(2):
                if nocc or nocc_rs:
                    nc.sync.dma_start(rs_out_h[h], rs_in_h[h][0:D_MODEL, :])
                else:
                    nc.gpsimd.collective_compute("ReduceScatter", ALU.add,
                                                 replica_groups=g4,
                                                 ins=[rs_in_h[h]], outs=[rs_out_h[h]])

        mid.close()

        # ============ P8: MLP tail ============
        with ExitStack() as p8:
            wmlp = p8.enter_context(tc.tile_pool(name="wmlp", bufs=1))
            w_fc_t = [wmlp.tile([128, 2 * D_MODEL], BF16, tag=f"wf{k}", name=f"wf{k}") for k in range(8)]
            for k in range(8):
                nc.sync.dma_start(w_fc_t[k][:], w_fc_T[128 * k:128 * (k + 1), :])
            w_pr_t = [wmlp.tile([128, D_MODEL], BF16, tag=f"wp{k}", name=f"wp{k}") for k in range(16)]
            for k in range(16):
                nc.sync.dma_start(w_pr_t[k][:], w_pr_T[128 * k:128 * (k + 1), :])

            ar = p8.enter_context(tc.tile_pool(name="p8a", bufs=1))
            st = p8.enter_context(tc.tile_pool(name="p8t", bufs=2))
            ppt = p8.enter_context(tc.tile_pool(name="p8pt", bufs=2, space="PSUM"))
            ppm = p8.enter_context(tc.tile_pool(name="p8pm", bufs=2, space="PSUM"))
            pp1 = p8.enter_context(tc.tile_pool(name="p8p1", bufs=1, space="PSUM"))

            TH = TQ // 2
            for th in range(2):
                t0 = TH * th
                x2_T = [ar.tile([128, TH], F32, tag=f"x2T{k}", name=f"x2T{k}")
                        for k in range(8)]
                for k in range(8):
                    nc.sync.dma_start(x2_T[k][:], xqT_dram[128 * k:128 * (k + 1), t0:t0 + TH])
                rsb = [ar.tile([128, TH], BF16, tag=f"rsb{k}", name=f"rsb{k}") for k in range(8)]
                for k in range(8):
                    nc.sync.dma_start(rsb[k][:], rs_out_h[th][128 * k:128 * (k + 1), :])
                    nc.vector.tensor_tensor(x2_T[k][:], x2_T[k][:], rsb[k][:], ALU.add)

                # rmsnorm over features via ones-matmul
                sq = [ar.tile([128, TH], BF16, tag=f"sq{k}", name=f"sq{k}") for k in range(8)]
                for k in range(8):
                    nc.scalar.activation(sq[k][:], x2_T[k][:], AF.Square)
                pss = pp1.tile([1, TH], F32, tag="pss", name="pss")
                for k in range(8):
                    nc.tensor.matmul(pss[:], ones_t[:], sq[k][:], start=(k == 0), stop=(k == 7))
                rrow = st.tile([1, TH], F32, tag="rrow", name="rrow")
                nc.scalar.activation(rrow[:], pss[:], AF.Sqrt, scale=1.0 / D_MODEL,
                                     bias=eps_t[0:1, 0:1])
                rrec = st.tile([1, TH], F32, tag="rrec", name="rrec")
                nc.vector.reciprocal(rrec[:], rrow[:])
                rbf = st.tile([1, TH], BF16, tag="rbf", name="rbf")
                nc.vector.tensor_copy(rbf[:], rrec[:])
                pr2 = pp1.tile([128, TH], F32, tag="pr2", name="pr2")
                nc.tensor.matmul(pr2[:], onesr_t[:], rbf[:], start=True, stop=True)
                x2n = [ar.tile([128, TH], BF16, tag=f"x2n{k}", name=f"x2n{k}") for k in range(8)]
                for k in range(8):
                    nc.vector.tensor_tensor(x2n[k][:], x2_T[k][:], pr2[:], ALU.mult)

                # c_fc + relu^2
                hh_t = [ar.tile([128, TH], BF16, tag=f"hh{k}", name=f"hh{k}") for k in range(16)]
                for m in range(16):
                    pm = ppm.tile([128, TH], F32, tag="pmm", name="pmm")
                    for k in range(8):
                        nc.tensor.matmul(pm[:], w_fc_t[k][:, 128 * m:128 * (m + 1)], x2n[k][:],
                                         start=(k == 0), stop=(k == 7))
                    rl = st.tile([128, TH], BF16, tag="rl", name="rl")
                    nc.scalar.activation(rl[:], pm[:], AF.Relu)
                    nc.vector.tensor_tensor(hh_t[m][:], rl[:], rl[:], ALU.mult)
                # c_proj + residual
                fin = [ar.tile([128, TH], F32, tag=f"fin{k}", name=f"fin{k}") for k in range(8)]
                for m in range(8):
                    pm = ppm.tile([128, TH], F32, tag="pmm", name="pmm")
                    for k in range(16):
                        nc.tensor.matmul(pm[:], w_pr_t[k][:, 128 * m:128 * (m + 1)], hh_t[k][:],
                                         start=(k == 0), stop=(k == 15))
                    nc.vector.tensor_tensor(fin[m][:], x2_T[m][:], pm[:], ALU.add)
                # transpose to token-major + store
                for i in range(TH // 128):
                    for h in range(2):
                        pt = ppt.tile([128, 512], F32, tag="ptx", name="ptx")
                        for q in range(4):
                            m = 4 * h + q
                            nc.tensor.transpose(pt[:, 128 * q:128 * (q + 1)],
                                                fin[m][:, 128 * i:128 * (i + 1)], idf_t[:])
                        ot = st.tile([128, 512], F32, tag="ot", name="ot")
                        nc.scalar.copy(ot[:], pt[:])
                        nc.sync.dma_start(out[t0 + 128 * i:t0 + 128 * (i + 1),
                                              512 * h:512 * (h + 1)], ot[:])

    nc.compile()
    return nc


def _prep_inputs(inputs):
    x = np.asarray(inputs['x'], np.float32)
    in_proj_w = np.asarray(inputs['in_proj_w'], np.float32)
    conv_w = np.asarray(inputs['conv_w'], np.float32)
    conv_b = np.asarray(inputs['conv_b'], np.float32)
    x_proj_w = np.asarray(inputs['x_proj_w'], np.float32)
    dt_proj_w = np.asarray(inputs['dt_proj_w'], np.float32)
    dt_proj_b = np.asarray(inputs['dt_proj_b'], np.float32)
    A_log = np.asarray(inputs['A_log'], np.float32)
    D = np.asarray(inputs['D'], np.float32)
    out_proj_w = np.asarray(inputs['out_proj_w'], np.float32)
    c_fc_w = np.asarray(inputs['c_fc_w'], np.float32)
    c_proj_w = np.asarray(inputs['c_proj_w'], np.float32)

    import ml_dtypes
    bf = lambda a: np.ascontiguousarray(a).astype(ml_dtypes.bfloat16)
    f32 = lambda a: np.ascontiguousarray(a, np.float32)

    r01 = np.zeros((16, 128, 128), np.float32)  # [jm][k, m] = 1 iff k == 8*jm + m%8
    g01 = np.zeros((16, 128, 128), np.float32)  # [jm][k, m] = 1 iff m == 8*jm + k%8
    for jm in range(16):
        for m in range(128):
            r01[jm, 8 * jm + (m % 8), m] = 1.0
            g01[jm, m, 8 * jm + (m % 8)] = 1.0
    s01n = np.zeros((D_STATE, 128), np.float32)
    s01p = np.zeros((D_STATE, 128), np.float32)
    for m in range(128):
        s01n[m // 8, m] = -1.0
        s01p[m // 8, m] = 1.0
    ident = np.eye(128, dtype=np.float32)

    def col_fold(a):
        # (EL,) or (EL, w) -> (128, NET*w): cols [w*k:w*(k+1)] = rows of e-tile k
        a = a.reshape(EL, -1)
        w = a.shape[1]
        o = np.zeros((128, NET * w), np.float32)
        for k in range(NET):
            o[:, w * k:w * (k + 1)] = a[128 * k:128 * (k + 1)]
        return o

    in_maps = []
    for c in range(8):
        b, r = c // 4, c % 4
        sl = slice(EL * r, EL * (r + 1))
        negA_ = np.zeros((128, NJ), np.float32)
        p = np.arange(128)
        for j in range(NJ):
            e = EL * r + 8 * j + (p % 8)
            s = p // 8
            negA_[:, j] = -np.exp(A_log[e, s])
        d_diag_ = np.zeros((128, NET * 128), np.float32)
        for k in range(NET):
            d_diag_[:, 128 * k:128 * (k + 1)] = np.diag(D[sl][128 * k:128 * (k + 1)])
        msk0 = np.full((XD, 1), 1.0 if b == 0 else 0.0, np.float32)
        msk1 = np.full((XD, 1), 1.0 if b == 1 else 0.0, np.float32)
        in_maps.append({
            'xb': f32(x[b]),
            'xq': f32(x[b][TQ * r:TQ * (r + 1)]),
            'w_in_T': bf(np.concatenate([in_proj_w[sl], in_proj_w[D_INNER:][sl]], 0).T),
            'conv_wc': col_fold(conv_w[sl]),
            'conv_bc': col_fold(conv_b[sl]),
            'w_xp_T': bf(x_proj_w[:, sl].T),
            'w_dt_T': bf(dt_proj_w[sl].T),
            'dt_bnc': col_fold(dt_proj_b[sl]),
            'negA': negA_,
            'd_c': col_fold(D[sl]),
            'd_diag': bf(d_diag_),
            'w_out_T': bf(out_proj_w[:, sl].T),
            'w_fc_T': bf(c_fc_w.T),
            'w_pr_T': bf(c_proj_w.T),
            'r01': bf(r01.reshape(16 * 128, 128)),
            'g01': bf(g01.reshape(16 * 128, 128)),
            's01n': bf(s01n),
            's01p': bf(s01p),
            'ident_bf': bf(ident),
            'ident_f32': f32(ident),
            'ones_bf': bf(np.ones((128, 1), np.float32)),
            'ones_row_bf': bf(np.ones((1, 128), np.float32)),
            'msk0': msk0,
            'msk1': msk1,
        })
    return in_maps


def kernel(**inputs) -> np.ndarray:
    if 'nc' not in _CACHE:
        _CACHE['nc'] = _build()
    nc = _CACHE['nc']
    in_maps = _prep_inputs(inputs)
    res = run_bass_kernel_spmd(nc, in_maps, core_ids=list(range(8)))
    out = np.zeros((B, T, D_MODEL), np.float32)
    for c in range(8):
        b, r = c // 4, c % 4
        out[b, TQ * r:TQ * (r + 1), :] = res.results[c]['out']
    return out



# revision 12
# speedup vs baseline: 1.4598x; 1.2145x over previous
"""Trainium2 Bass kernel for nn_Block_4526895530469 (Mamba block + MLP residual).

v2: two time-chunk (1024-token) pipelined design.

Sharding over 8 NeuronCores: core c -> batch b=c//4, channel shard r=c%4
(512 of 2048 d_inner channels). Each core also owns 2x256 output tokens
(one 256-slice per time-chunk) for the token-parallel MLP tail.

Pipeline per chunk: in_proj -> conv -> x_proj -> AllReduce8 -> dt-path ->
selective scan (DVE) -> out_proj -> ReduceScatter4 (scatters 256-token
slices). Chunk B's front section fills chunk A's AllReduce window; RS of
chunk A hides under chunk B's scan.

Key engine choices (from trace analysis):
- All elementwise on DVE; GpSimd elementwise locks the shared DVE/GpSimd
  SBUF port pair and slows DVE 2-3x, so GpSimd only issues DMAs/collectives.
- rmsnorm in feature-major from a host-transposed x: sum x^2 via
  ones-matmul, rsqrt = exp(-0.5*ln(.)) so all ACT ops share one table set
  (natural_log_exp_and_others) with the scan exps; softplus = ln(1+exp).
- conv1d and the u*D skip run as diagonal-weight matmuls accumulated in
  PSUM; biases ride the ACT ops.
- No transposes on device: host provides x feature-major; output returns
  feature-major and is transposed in numpy.
"""
import sys
sys.path.insert(0, '/opt/trn_rl_repo')

import numpy as np
from contextlib import ExitStack

import concourse.bass as bass
from concourse import bacc
import concourse.tile as tile
from concourse import mybir
from concourse.bass_utils import run_bass_kernel_spmd

# The interp (used by Tile's scheduling pass and test simulation) lacks
# Silu; emulate via the Sigmoid path.
from concourse import bass_interp as _bi
from concourse import mybir as _mb

_orig_visit_act = _bi.InstructionExecutor.visit_InstActivation


def _visit_act_with_silu(self, instruction, *a, **kw):
    if instruction.func != _mb.ActivationFunctionType.Silu:
        return _orig_visit_act(self, instruction, *a, **kw)
    import numpy as _np
    assert len(instruction.outs) == 1, "Silu shim: no accum_out support"
    func0 = instruction.func
    try:
        instruction.func = _mb.ActivationFunctionType.Sigmoid
        res = _orig_visit_act(self, instruction, *a, **kw)
    finally:
        instruction.func = func0
    reg_snapshot = kw.get("reg_snapshot")
    inp = self.view_ap(instruction.ins[0], _bi.Direction.READ, instruction,
                       reg_snapshot=reg_snapshot).astype(_np.float32)
    inp = inp.reshape(inp.shape[0], -1)

    def _val(arg):
        if isinstance(arg, _mb.ImmediateValue):
            return arg.value
        v = self.view_ap(arg, _bi.Direction.READ, instruction,
                         reg_snapshot=reg_snapshot).astype(_np.float32)
        return v.reshape(v.shape[0], -1)

    bias = _val(instruction.ins[1])
    scale = _val(instruction.ins[2])
    sx = inp * scale + bias
    out_view = self.view_ap(instruction.outs[0], _bi.Direction.WRITE, instruction,
                            reg_snapshot=reg_snapshot)
    sig = _np.asarray(out_view, dtype=_np.float32).reshape(sx.shape)
    out_view[:] = (sig * sx).reshape(out_view.shape).astype(out_view.dtype)
    return res


_bi.InstructionExecutor.visit_InstActivation = _visit_act_with_silu

# The act-table-load pass resolves each activation to the FIRST table set
# containing it: exp -> exp_and_others, ln -> natural_log. Our kernel
# interleaves exp and ln constantly (softplus, rsqrt-via-exp/ln), which
# would reload tables on every transition (~2.6us each). Restrict exp/ln
# to the combined natural_log_exp_and_others set, keeping list positions
# (ids index into act_info.json's act_func_sets) intact.
import concourse.bacc as _bacc
_orig_get_tables = _bacc.get_activation_tables


def _tables_prefer_combined(arch):
    t = dict(_orig_get_tables(arch))
    for name, fns in t.items():
        if name == 'natural_log_exp_and_others':
            continue
        fns = set(fns)
        fns.discard(_mb.ActivationFunctionType.Exp)
        fns.discard(_mb.ActivationFunctionType.Ln)
        t[name] = fns
    return t


_bacc.get_activation_tables = _tables_prefer_combined

F32 = mybir.dt.float32
BF16 = mybir.dt.bfloat16
AF = mybir.ActivationFunctionType
ALU = mybir.AluOpType

D_MODEL, D_INNER, D_STATE, D_CONV, DT_RANK = 1024, 2048, 16, 4, 64
B, T = 2, 2048
EL = D_INNER // 4          # 512 channels per core
NET = EL // 128            # 4 e-tiles
NJ = EL // 8               # 64 scan tiles
HC = T // 2                # chunk width (1024)
TS = 256                   # tokens per (core, chunk) in the MLP tail
XD = DT_RANK + 2 * D_STATE  # 96
EPS = float(np.finfo(np.float32).eps)

_CACHE = {}


def _build():
    nc = bacc.Bacc("TRN2", target_bir_lowering=False, debug=False, num_devices=8)

    def din(name, shape, dt=BF16):
        return nc.dram_tensor(name, list(shape), dt, kind="ExternalInput").ap()

    xbT = din("xbT", (D_MODEL, T))                       # bf16 feature-major x
    xqT = din("xqT", (D_MODEL, 2 * TS))                  # core's tokens, f-major bf16
    w_in_T = din("w_in_T", (D_MODEL, 2 * EL))
    conv_wc = din("conv_wc", (128, NET * D_CONV), F32)   # cols [4k:4k+4] = e-tile k
    conv_bc = din("conv_bc", (128, NET), F32)
    w_xp_T = din("w_xp_T", (EL, XD))
    w_dt_T = din("w_dt_T", (DT_RANK, EL))
    dt_bnc = din("dt_bnc", (128, NET), F32)              # +dt_proj_b
    negA = din("negA", (128, NJ), F32)                   # A (negative)
    d_diag = din("d_diag", (128, NET * 128))             # diag(D) per e-tile
    w_out_T = din("w_out_T", (EL, D_MODEL))
    w_fc_T = din("w_fc_T", (D_MODEL, 2 * D_MODEL))
    w_pr_T = din("w_pr_T", (2 * D_MODEL, D_MODEL))
    g01 = din("g01", (16 * 128, 128))
    s01p = din("s01p", (D_STATE, 128))
    ones_bf = din("ones_bf", (128, 1))
    ones_row_bf = din("ones_row_bf", (1, 128))

    out = nc.dram_tensor("out", [D_MODEL, 2 * TS], F32, kind="ExternalOutput").ap()

    xdbl_in = [nc.dram_tensor(f"xdbl_in{c}", [XD, HC], BF16).ap() for c in range(2)]
    xdbl_out = [nc.dram_tensor(f"xdbl_out{c}", [XD, HC], BF16).ap() for c in range(2)]
    rs_in = [nc.dram_tensor(f"rs_in{c}", [4 * D_MODEL, TS], BF16).ap() for c in range(2)]
    rs_out = [nc.dram_tensor(f"rs_out{c}", [D_MODEL, TS], BF16).ap() for c in range(2)]
    dtu_dram = [[nc.dram_tensor(f"dtu_dram{c}_{m}", [128, HC], BF16).ap()
                 for m in range(NET)] for c in range(2)]
    dt_dram = [[nc.dram_tensor(f"dt_dram{c}_{m}", [128, HC], BF16).ap()
                for m in range(NET)] for c in range(2)]
    sz_dram = [nc.dram_tensor(f"sz_dram{c}", [EL, HC], BF16).ap() for c in range(2)]

    g8 = [[0, 1, 2, 3, 4, 5, 6, 7]]
    g4 = [[0, 1, 2, 3], [4, 5, 6, 7]]

    with tile.TileContext(nc) as tc, ExitStack() as top:
        cpool = top.enter_context(tc.tile_pool(name="consts", bufs=1))

        def cload(nm, name_ap, shape, dt=BF16):
            t = cpool.tile(list(shape), dt, tag=nm, name=nm)
            nc.scalar.dma_start(t[:], name_ap)
            return t

        negA_t = cload("negA_t", negA, (128, NJ), F32)
        convw_t = cload("convw_t", conv_wc, (128, NET * D_CONV), F32)
        convb_t = cload("convb_t", conv_bc, (128, NET), F32)
        dtbn_t = cload("dtbn_t", dt_bnc, (128, NET), F32)
        ddiag_t = cload("ddiag_t", d_diag, (128, NET * 128))
        s01p_t = cload("s01p_t", s01p, (D_STATE, 128))
        ones_t = cload("ones_t", ones_bf, (128, 1))
        onesr_t = cload("onesr_t", ones_row_bf, (1, 128))
        g01_t = cpool.tile([128, 16 * 128], BF16)
        for k in range(16):
            nc.scalar.dma_start(g01_t[:, 128 * k:128 * (k + 1)], g01[128 * k:128 * (k + 1), :])
        eps_t = cpool.tile([128, 1], F32)
        nc.vector.memset(eps_t[:], EPS)
        one_t = cpool.tile([128, 1], F32)
        nc.vector.memset(one_t[:], 1.0)

        # PSUM pools (bank budget 8 = 16KB/partition):
        # prp 2x[128,1024] = 4, pyp 1x[128,1024] = 2, mmp 2x[128,512] = 2.
        # mmp hands out a single [128,512] f32 tag; callers slice views.
        mmp = top.enter_context(tc.tile_pool(name="mmp", bufs=2, space="PSUM"))

        def mm():
            return mmp.tile([128, 512], F32, tag="mm", name="mm")

        sp = top.enter_context(tc.tile_pool(name="sp", bufs=1))      # small staging
        dbp = top.enter_context(tc.tile_pool(name="dbp", bufs=2))    # dbl-buffered staging

        # mid-lived activations (freed before the MLP tail)
        mid = top.enter_context(ExitStack())
        acts = mid.enter_context(tc.tile_pool(name="acts", bufs=1))
        u_bf = [acts.tile([128, T], BF16, tag=f"u{k}", name=f"u{k}") for k in range(NET)]
        brep_t = acts.tile([128, T], BF16, tag="brep", name="brep")
        crep_t = acts.tile([128, T], BF16, tag="crep", name="crep")
        hc_t = acts.tile([128, NJ], F32, tag="hc", name="hc")

        wmid = mid.enter_context(tc.tile_pool(name="wmid", bufs=1))
        w_xp_t = [wmid.tile([128, XD], BF16, tag=f"wxp{k}", name=f"wxp{k}") for k in range(NET)]
        for k in range(NET):
            nc.scalar.dma_start(w_xp_t[k][:], w_xp_T[128 * k:128 * (k + 1), :])
        w_dt_t = wmid.tile([DT_RANK, EL], BF16, tag="wdt", name="wdt")
        nc.scalar.dma_start(w_dt_t[:], w_dt_T)
        w_out_t = [wmid.tile([128, D_MODEL], BF16, tag=f"wo{k}", name=f"wo{k}") for k in range(NET)]
        for k in range(NET):
            nc.scalar.dma_start(w_out_t[k][:], w_out_T[128 * k:128 * (k + 1), :])

        # MLP pools open below mid-level pools so they survive scn.close()
        mlpa = top.enter_context(tc.tile_pool(name="mlpa", bufs=1))
        mlp_s = top.enter_context(tc.tile_pool(name="mlps", bufs=2))
        mlpw1 = top.enter_context(tc.tile_pool(name="mlpw1", bufs=1))

        # front-phase-only allocations (closed before the scan pools open)
        fr = top.enter_context(ExitStack())
        frw = fr.enter_context(tc.tile_pool(name="frw", bufs=1))
        w_in_t = [frw.tile([128, 2 * EL], BF16, tag=f"wi{k}", name=f"wi{k}") for k in range(8)]
        for k in range(8):
            nc.scalar.dma_start(w_in_t[k][:], w_in_T[128 * k:128 * (k + 1), :])
        x_in = [frw.tile([128, T + 4], BF16, tag=f"xi{k}", name=f"xi{k}") for k in range(NET)]
        for k in range(NET):
            nc.vector.memset(x_in[k][:, 0:4], 0.0)
        frp = fr.enter_context(tc.tile_pool(name="frp", bufs=2))
        xbpool = fr.enter_context(tc.tile_pool(name="xbp", bufs=8))
        sqpool = fr.enter_context(tc.tile_pool(name="sqp", bufs=2))
        xnpool = fr.enter_context(tc.tile_pool(name="xnp", bufs=8))

        def front(c):
            """rmsnorm + in_proj + conv + x_proj + AR for chunk c."""
            lo = HC * c
            # ---- feature-major rmsnorm ----
            xb_t = []
            for k in range(8):
                xt = xbpool.tile([128, HC], BF16, tag="xb", name="xb")
                q = nc.sync if k % 2 == 0 else nc.scalar
                q.dma_start(xt[:], xbT[128 * k:128 * (k + 1), lo:lo + HC])
                xb_t.append(xt)
            rr = frp.tile([1, HC], BF16, tag="rr", name="rr")
            pssh = [mm() for _ in range(2)]
            for k in range(8):
                sq = sqpool.tile([128, HC], BF16, tag="sq", name="sq")
                nc.scalar.activation(sq[:], xb_t[k][:], AF.Square)
                for h in range(2):
                    nc.tensor.matmul(pssh[h][0:1, :], ones_t[:],
                                     sq[:, 512 * h:512 * (h + 1)],
                                     start=(k == 0), stop=(k == 7))
            for h in range(2):
                rln = frp.tile([1, 512], F32, tag="rln", name="rln")
                nc.scalar.activation(rln[:], pssh[h][0:1, :], AF.Ln,
                                     scale=1.0 / D_MODEL, bias=eps_t[0:1, 0:1])
                nc.scalar.activation(rr[:, 512 * h:512 * (h + 1)], rln[:], AF.Exp,
                                     scale=-0.5)
            rrep = frp.tile([128, HC], BF16, tag="rrep", name="rrep")
            for h in range(2):
                prb = mm()
                nc.tensor.matmul(prb[:], onesr_t[:], rr[:, 512 * h:512 * (h + 1)],
                                 start=True, stop=True)
                nc.scalar.copy(rrep[:, 512 * h:512 * (h + 1)], prb[:])
            xn_t = []
            for k in range(8):
                xn = xnpool.tile([128, HC], BF16, tag="xn", name="xn")
                nc.vector.tensor_tensor(xn[:], xb_t[k][:], rrep[:], ALU.mult)
                xn_t.append(xn)
            # ---- in_proj, x-half first (z-half after the AR fires) ----
            def proj_m(m):
                for h in range(2):
                    ps = mm()
                    for k in range(8):
                        nc.tensor.matmul(ps[:], w_in_t[k][:, 128 * m:128 * (m + 1)],
                                         xn_t[k][:, 512 * h:512 * (h + 1)],
                                         start=(k == 0), stop=(k == 7))
                    if m < 4:
                        nc.scalar.copy(x_in[m][:, 4 + lo + 512 * h:4 + lo + 512 * (h + 1)],
                                       ps[:])
                    else:
                        szt = frp.tile([128, 512], BF16, tag="szt", name="szt")
                        nc.scalar.activation(szt[:], ps[:], AF.Silu)
                        nc.sync.dma_start(
                            sz_dram[c][128 * (m - 4):128 * (m - 3),
                                       512 * h:512 * (h + 1)], szt[:])
            for m in range(4):
                proj_m(m)
                j = m
                xc = frp.tile([128, HC], BF16, tag="xc", name="xc")
                nc.vector.tensor_scalar(xc[:], x_in[j][:, 4 + lo:4 + lo + HC],
                                        convw_t[:, 4 * j + 3:4 * j + 4],
                                        convb_t[:, j:j + 1], ALU.mult, ALU.add)
                for sh in range(1, 4):
                    nc.vector.scalar_tensor_tensor(
                        xc[:], x_in[j][:, 4 + lo - sh:4 + lo + HC - sh],
                        convw_t[:, 4 * j + 3 - sh:4 * j + 4 - sh],
                        xc[:], ALU.mult, ALU.add)
                nc.scalar.activation(u_bf[j][:, lo:lo + HC], xc[:], AF.Silu)
            # ---- x_proj partial + masked AR staging ----
            t0 = frp.tile([XD, HC], BF16, tag="t0", name="t0")
            for h in range(2):
                px = mm()
                for k in range(NET):
                    nc.tensor.matmul(px[0:XD, :], w_xp_t[k][:],
                                     u_bf[k][:, lo + 512 * h:lo + 512 * (h + 1)],
                                     start=(k == 0), stop=(k == NET - 1))
                nc.scalar.copy(t0[:, 512 * h:512 * (h + 1)], px[0:XD, :])
            nc.sync.dma_start(xdbl_in[c][:], t0[:])
            # per-batch reduction: groups {0..3}, {4..7} reduce independently
            nc.gpsimd.collective_compute("AllReduce", ALU.add, replica_groups=g4,
                                         ins=[xdbl_in[c]], outs=[xdbl_out[c]])
            # z-half of in_proj fills the AllReduce window
            for m in range(4, 8):
                proj_m(m)

        def dt_path(c):
            """B/C broadcast, dt softplus, dtu/dt staging for chunk c."""
            lo = HC * c
            dl = sp.tile([DT_RANK, HC], BF16, tag="dl", name="dl")
            nc.sync.dma_start(dl[:], xdbl_out[c][0:DT_RANK, :])
            b_sb = sp.tile([D_STATE, HC], BF16, tag="bsb", name="bsb")
            nc.sync.dma_start(b_sb[:], xdbl_out[c][DT_RANK:DT_RANK + D_STATE, :])
            c_sb = sp.tile([D_STATE, HC], BF16, tag="csb", name="csb")
            nc.sync.dma_start(c_sb[:], xdbl_out[c][DT_RANK + D_STATE:XD, :])
            for h in range(2):
                pb = mm()
                nc.tensor.matmul(pb[:], s01p_t[:],
                                 b_sb[:, 512 * h:512 * (h + 1)], start=True, stop=True)
                nc.scalar.copy(brep_t[:, lo + 512 * h:lo + 512 * (h + 1)], pb[:])
                pcr = mm()
                nc.tensor.matmul(pcr[:], s01p_t[:],
                                 c_sb[:, 512 * h:512 * (h + 1)], start=True, stop=True)
                nc.scalar.copy(crep_t[:, lo + 512 * h:lo + 512 * (h + 1)], pcr[:])
            for m in range(NET):
                e1 = e1p.tile([128, HC], BF16, tag="e1", name="e1")
                for h in range(2):
                    pd = mm()
                    nc.tensor.matmul(pd[:],
                                     w_dt_t[:, 128 * m:128 * (m + 1)],
                                     dl[:, 512 * h:512 * (h + 1)], start=True, stop=True)
                    # softplus(pd+b) = ln(1 + exp(pd+b)); exp/ln share a set
                    nc.scalar.activation(e1[:, 512 * h:512 * (h + 1)], pd[:], AF.Exp,
                                         bias=dtbn_t[:, m:m + 1])
                dtl = e1p.tile([128, HC], BF16, tag="dtl", name="dtl")
                nc.scalar.activation(dtl[:], e1[:], AF.Ln, bias=one_t[:, 0:1])
                nc.sync.dma_start(dt_dram[c][m][:], dtl[:])
                dtu = dbp.tile([128, HC], BF16, tag="dtu", name="dtu")
                nc.vector.tensor_tensor(dtu[:], dtl[:], u_bf[m][:, lo:lo + HC], ALU.mult)
                nc.sync.dma_start(dtu_dram[c][m][:], dtu[:])

        def scan_chunk(c, Js, y2c):
            """scan tiles for J in Js over chunk c; appends y2 tiles to y2c."""
            lo = HC * c
            for J in Js:
                py = pyp.tile([128, HC], F32, tag="py", name="py")
                for jj in range(16):
                    j = 16 * J + jj
                    dtr = drp.tile([128, HC], BF16, tag="dtr", name="dtr")
                    srd = dt_dram[c][J][8 * jj:8 * jj + 8, :]
                    nc.gpsimd.dma_start(dtr[:], srd.unsqueeze(0).broadcast_to([16, 8, HC]))
                    dA = dap.tile([128, HC], BF16, tag="dA", name="dA")
                    nc.scalar.activation(dA[:], dtr[:], AF.Exp, scale=negA_t[:, j:j + 1])
                    dtur = dtp.tile([128, HC], BF16, tag="dtur", name="dtur")
                    src = dtu_dram[c][J][8 * jj:8 * jj + 8, :]
                    nc.gpsimd.dma_start(dtur[:], src.unsqueeze(0).broadcast_to([16, 8, HC]))
                    bb = bbp.tile([128, HC], BF16, tag="bb", name="bb")
                    nc.vector.tensor_tensor(bb[:], dtur[:], brep_t[:, lo:lo + HC], ALU.mult)
                    hh = hhp.tile([128, HC], BF16, tag="hh", name="hh")
                    init = 0.0 if c == 0 else hc_t[:, j:j + 1]
                    nc.vector.tensor_tensor_scan(hh[:], dA[:], bb[:], init,
                                                 ALU.mult, ALU.add)
                    if c == 0:
                        nc.scalar.copy(hc_t[:, j:j + 1], hh[:, HC - 1:HC])
                    t1 = t1p.tile([128, HC], BF16, tag="t1", name="t1")
                    nc.vector.tensor_tensor(t1[:], hh[:], crep_t[:, lo:lo + HC], ALU.mult)
                    for h in range(2):
                        nc.tensor.matmul(py[:, 512 * h:512 * (h + 1)],
                                         g01_t[:, 128 * jj:128 * (jj + 1)],
                                         t1[:, 512 * h:512 * (h + 1)],
                                         start=(jj == 0), stop=False)
                for h in range(2):
                    nc.tensor.matmul(py[:, 512 * h:512 * (h + 1)],
                                     ddiag_t[:, 128 * J:128 * (J + 1)],
                                     u_bf[J][:, lo + 512 * h:lo + 512 * (h + 1)],
                                     start=False, stop=True)
                szl = e1p.tile([128, HC], BF16, tag="szl", name="szl")
                nc.sync.dma_start(szl[:], sz_dram[c][128 * J:128 * (J + 1), :])
                y2 = y2p.tile([128, HC], BF16, tag=f"y2_{J}", name=f"y2_{J}")
                nc.vector.tensor_tensor(y2[:], py[:], szl[:], ALU.mult)
                y2c.append(y2)

        def out_proj(c, y2c):
            for m in range(8):
                for h in range(2):
                    po = mm()
                    for k in range(NET):
                        nc.tensor.matmul(po[:], w_out_t[k][:, 128 * m:128 * (m + 1)],
                                         y2c[k][:, 512 * h:512 * (h + 1)],
                                         start=(k == 0), stop=(k == NET - 1))
                    ob = dbp.tile([128, 512], BF16, tag="ob", name="ob")
                    nc.scalar.copy(ob[:], po[:])
                    for q in range(2):
                        k = 2 * h + q
                        nc.sync.dma_start(
                            rs_in[c][D_MODEL * k + 128 * m:D_MODEL * k + 128 * (m + 1), :],
                            ob[:, TS * q:TS * (q + 1)])
            nc.gpsimd.collective_compute("ReduceScatter", ALU.add, replica_groups=g4,
                                         ins=[rs_in[c]], outs=[rs_out[c]])

        # ================= emission =================
        mlp_state = {}

        def mlp_preload():
            w_fc_t = [mlpw1.tile([128, 2 * D_MODEL], BF16, tag=f"wf{k}", name=f"wf{k}")
                      for k in range(8)]
            for k in range(8):
                nc.sync.dma_start(w_fc_t[k][:], w_fc_T[128 * k:128 * (k + 1), :])
            mlp_state['wfc'] = w_fc_t
            xq_t = [mlpw1.tile([128, 2 * TS], BF16, tag=f"xqp{k}", name=f"xqp{k}")
                    for k in range(8)]
            for k in range(8):
                nc.scalar.dma_start(xq_t[k][:], xqT[128 * k:128 * (k + 1), :])
            mlp_state['xq'] = xq_t

        def mlp_fc(c):
            w_fc_t = mlp_state['wfc']
            xq_t = mlp_state['xq']
            x2 = [mlpa.tile([128, TS], BF16, tag=f"x2_{k}", name=f"x2_{k}")
                  for k in range(8)]
            pssm = mm()
            pss = pssm[0:1, 0:TS]
            for k in range(8):
                rsb = mlp_s.tile([128, TS], BF16, tag="rsb", name="rsb")
                nc.sync.dma_start(rsb[:], rs_out[c][128 * k:128 * (k + 1), :])
                nc.vector.tensor_tensor(x2[k][:], xq_t[k][:, TS * c:TS * (c + 1)],
                                        rsb[:], ALU.add)
                s = mlp_s.tile([128, TS], BF16, tag="sq8", name="sq8")
                nc.scalar.activation(s[:], x2[k][:], AF.Square)
                nc.tensor.matmul(pss, ones_t[:], s[:], start=(k == 0), stop=(k == 7))
            rln = mlp_s.tile([1, TS], F32, tag="rln8", name="rln8")
            nc.scalar.activation(rln[:], pss, AF.Ln, scale=1.0 / D_MODEL,
                                 bias=eps_t[0:1, 0:1])
            rr = mlp_s.tile([1, TS], BF16, tag="rr8", name="rr8")
            nc.scalar.activation(rr[:], rln[:], AF.Exp, scale=-0.5)
            prb = mm()
            nc.tensor.matmul(prb[:, 0:TS], onesr_t[:], rr[:], start=True, stop=True)
            rrep = mlp_s.tile([128, TS], BF16, tag="rrep8", name="rrep8")
            nc.scalar.copy(rrep[:], prb[:, 0:TS])
            x2n = []
            for k in range(8):
                xn = mlpa.tile([128, TS], BF16, tag=f"x2n_{k}", name=f"x2n_{k}")
                nc.vector.tensor_tensor(xn[:], x2[k][:], rrep[:], ALU.mult)
                x2n.append(xn)
            hh2 = []
            for m in range(16):
                pm = mm()
                for k in range(8):
                    nc.tensor.matmul(pm[:, 0:TS], w_fc_t[k][:, 128 * m:128 * (m + 1)],
                                     x2n[k][:], start=(k == 0), stop=(k == 7))
                rl = mlp_s.tile([128, TS], BF16, tag="rl8", name="rl8")
                nc.scalar.activation(rl[:], pm[:, 0:TS], AF.Relu)
                h2 = mlpa.tile([128, TS], BF16, tag=f"hh2_{m}", name=f"hh2_{m}")
                nc.vector.tensor_tensor(h2[:], rl[:], rl[:], ALU.mult)
                hh2.append(h2)
            mlp_state[c] = (x2, hh2)

        def mlp_proj(c):
            if 'wpr' not in mlp_state:
                w_pr_t = [mlpw2.tile([128, D_MODEL], BF16, tag=f"wp{k}", name=f"wp{k}")
                          for k in range(16)]
                for k in range(16):
                    nc.sync.dma_start(w_pr_t[k][:], w_pr_T[128 * k:128 * (k + 1), :])
                mlp_state['wpr'] = w_pr_t
            w_pr_t = mlp_state['wpr']
            x2, hh2 = mlp_state.pop(c)
            for m in range(8):
                pm = mm()
                for k in range(16):
                    nc.tensor.matmul(pm[:, 0:TS], w_pr_t[k][:, 128 * m:128 * (m + 1)],
                                     hh2[k][:], start=(k == 0), stop=(k == 15))
                fin = mlp_s.tile([128, TS], F32, tag="fin8", name="fin8")
                nc.vector.tensor_tensor(fin[:], x2[m][:], pm[:, 0:TS], ALU.add)
                nc.sync.dma_start(out[128 * m:128 * (m + 1), TS * c:TS * (c + 1)],
                                  fin[:])

        front(0)
        front(1)
        fr.close()

        scn = ExitStack()
        pyp = scn.enter_context(tc.tile_pool(name="pyp", bufs=2, space="PSUM"))
        pools = dict(
            dap=scn.enter_context(tc.tile_pool(name="dap", bufs=2)),
            drp=scn.enter_context(tc.tile_pool(name="drp", bufs=2)),
            dtp=scn.enter_context(tc.tile_pool(name="dtp", bufs=2)),
            bbp=scn.enter_context(tc.tile_pool(name="bbp", bufs=2)),
            hhp=scn.enter_context(tc.tile_pool(name="hhp", bufs=2)),
            t1p=scn.enter_context(tc.tile_pool(name="t1p", bufs=2)),
            y2p=scn.enter_context(tc.tile_pool(name="y2p", bufs=1)),
            e1p=scn.enter_context(tc.tile_pool(name="e1p", bufs=2)),
        )
        dap, dtp, bbp, drp = pools['dap'], pools['dtp'], pools['bbp'], pools['drp']
        hhp, t1p, y2p, e1p = pools['hhp'], pools['t1p'], pools['y2p'], pools['e1p']

        dt_path(0)
        dt_path(1)
        mlp_preload()
        y2a = []
        scan_chunk(0, range(4), y2a)
        out_proj(0, y2a)
        y2b = []
        scan_chunk(1, range(4), y2b)
        out_proj(1, y2b)

        scn.close()

        with ExitStack() as p8:
            mlpw2 = p8.enter_context(tc.tile_pool(name="mlpw2", bufs=1))
            # Pin the MLP tail after all scan/out_proj work in the scheduler's
            # timeline: the list scheduler otherwise hoists the RS-dependent
            # loads ahead of remaining scan tiles, stalling every queue behind
            # the in-flight collective.
            with tc.tile_wait_until(2.0):
                mlp_fc(0)               # fills the RS_1 window
                mlp_proj(0)
            with tc.tile_wait_until(2.1):
                mlp_fc(1)
                mlp_proj(1)

    nc.compile()
    return nc


def _prep_inputs(inputs):
    x = np.asarray(inputs['x'], np.float32)
    in_proj_w = np.asarray(inputs['in_proj_w'], np.float32)
    conv_w = np.asarray(inputs['conv_w'], np.float32)
    conv_b = np.asarray(inputs['conv_b'], np.float32)
    x_proj_w = np.asarray(inputs['x_proj_w'], np.float32)
    dt_proj_w = np.asarray(inputs['dt_proj_w'], np.float32)
    dt_proj_b = np.asarray(inputs['dt_proj_b'], np.float32)
    A_log = np.asarray(inputs['A_log'], np.float32)
    D = np.asarray(inputs['D'], np.float32)
    out_proj_w = np.asarray(inputs['out_proj_w'], np.float32)
    c_fc_w = np.asarray(inputs['c_fc_w'], np.float32)
    c_proj_w = np.asarray(inputs['c_proj_w'], np.float32)

    import ml_dtypes
    bf = lambda a: np.ascontiguousarray(a).astype(ml_dtypes.bfloat16)
    f32 = lambda a: np.ascontiguousarray(a, np.float32)

    r01 = np.zeros((16, 128, 128), np.float32)  # [jm][k, m] = 1 iff k == 8*jm + m%8
    g01 = np.zeros((16, 128, 128), np.float32)  # [jm][k, m] = 1 iff m == 8*jm + k%8
    for jm in range(16):
        for m in range(128):
            r01[jm, 8 * jm + (m % 8), m] = 1.0
            g01[jm, m, 8 * jm + (m % 8)] = 1.0
    s01p = np.zeros((D_STATE, 128), np.float32)
    for m in range(128):
        s01p[m // 8, m] = 1.0

    def col_fold(a):
        a = a.reshape(EL, -1)
        w = a.shape[1]
        o = np.zeros((128, NET * w), np.float32)
        for k in range(NET):
            o[:, w * k:w * (k + 1)] = a[128 * k:128 * (k + 1)]
        return o

    in_maps = []
    for c in range(8):
        b, r = c // 4, c % 4
        sl = slice(EL * r, EL * (r + 1))
        negA_ = np.zeros((128, NJ), np.float32)
        p = np.arange(128)
        for j in range(NJ):
            e = EL * r + 8 * j + (p % 8)
            s = p // 8
            negA_[:, j] = -np.exp(A_log[e, s])
        d_diag_ = np.zeros((128, NET * 128), np.float32)
        for jt in range(NET):
            d_diag_[:, 128 * jt:128 * (jt + 1)] = np.diag(D[sl][128 * jt:128 * (jt + 1)])
        tok = np.concatenate([np.arange(TS * r, TS * (r + 1)),
                              np.arange(HC + TS * r, HC + TS * (r + 1))])
        in_maps.append({
            'xbT': bf(x[b].T),
            'xqT': bf(x[b][tok].T),
            'w_in_T': bf(np.concatenate([in_proj_w[sl], in_proj_w[D_INNER:][sl]], 0).T),
            'conv_wc': col_fold(conv_w[sl]),
            'conv_bc': col_fold(conv_b[sl]),
            'w_xp_T': bf(x_proj_w[:, sl].T),
            'w_dt_T': bf(dt_proj_w[sl].T),
            'dt_bnc': col_fold(dt_proj_b[sl]),
            'negA': negA_,
            'd_diag': bf(d_diag_),
            'w_out_T': bf(out_proj_w[:, sl].T),
            'w_fc_T': bf(c_fc_w.T),
            'w_pr_T': bf(c_proj_w.T),
            'g01': bf(g01.reshape(16 * 128, 128)),
            's01p': bf(s01p),
            'ones_bf': bf(np.ones((128, 1), np.float32)),
            'ones_row_bf': bf(np.ones((1, 128), np.float32)),
        })
    return in_maps


def kernel(**inputs) -> np.ndarray:
    if 'nc' not in _CACHE:
        _CACHE['nc'] = _build()
    nc = _CACHE['nc']
    in_maps = _prep_inputs(inputs)
    res = run_bass_kernel_spmd(nc, in_maps, core_ids=list(range(8)))
    out = np.zeros((B, T, D_MODEL), np.float32)
    for c in range(8):
        b, r = c // 4, c % 4
        o = res.results[c]['out'].T          # [512, D_MODEL]
        out[b, TS * r:TS * (r + 1), :] = o[:TS]
        out[b, HC + TS * r:HC + TS * (r + 1), :] = o[TS:]
    return out
